# revision 1
# baseline (speedup 1.0000x reference)
"""Trainium2 Bass kernel for nn_EntityEncoder (language-adapter + BiLSTM + proj).

Sharding: 8 cores = 4 batch-quarters x 2 LSTM directions.
Each core: adapters for its 8 batch items, input projection for its
direction, 256-step LSTM recurrence, partial output projection.
Host sums fwd+bwd projection partials (concat @ Wp == sum of halves).
Backward direction realized by host-side time reversal of per-core inputs.

Layouts:
  phase 1 runs feature-major (h1T/h2T: [feat, token]) so LayerNorm stats are
  matmul reductions and no on-device transposes are needed; z lands
  token-major in DRAM for the recurrence.
  phase 2 keeps gates in a batch-major PSUM tile; gate columns are
  pre-permuted into 4 unit-blocks of [i|f|o|g]x128 so per-block elementwise
  starts when its PSUM bank completes.  h' is PE-transposed into the
  SBUF-resident unit-major ysT, which doubles as next step's stationary
  matmul operand and phase 3's input.
"""

import os

import numpy as np

B, S, H, HL, E, L = 32, 256, 1024, 512, 256, 5
G = 4 * HL            # 2048 gate width
NCORES = 8
BC = 8                # batch items per core
TOK = BC * S          # tokens per core
EPS = 1e-5
P = 128

_CACHE = {}
LAST_RUN = {}


def _gate_perm():
    """Permutation of the [i|f|g|o] gate axis into 4 unit-blocks of
    [i_n | f_n | o_n | g_n] (128 units each)."""
    perm = np.zeros(G, dtype=np.int64)
    for n in range(4):
        u = np.arange(128) + n * 128
        base = n * 512
        perm[base + 0 * 128: base + 1 * 128] = 0 * HL + u   # i
        perm[base + 1 * 128: base + 2 * 128] = 1 * HL + u   # f
        perm[base + 2 * 128: base + 3 * 128] = 3 * HL + u   # o
        perm[base + 3 * 128: base + 4 * 128] = 2 * HL + u   # g
    return perm


def _build_nc(use_mask, nsteps=S, phases=(1, 2, 3), parts='abcde'):
    import concourse.tile as tile
    import concourse.mybir as mybir
    from concourse import bacc

    dt = mybir.dt
    f32 = dt.float32
    f32r = dt.float32r
    AF = mybir.ActivationFunctionType
    ALU = mybir.AluOpType

    nc = bacc.Bacc(
        "TRN2", target_bir_lowering=False, debug=False, num_devices=NCORES
    )

    # ---------------- I/O ----------------
    xT = nc.dram_tensor("xT", [H, TOK], f32r, kind="ExternalInput").ap()
    W1s = nc.dram_tensor("W1s", [BC, H, H], f32r, kind="ExternalInput").ap()
    W2s = nc.dram_tensor("W2s", [BC, H, H], f32r, kind="ExternalInput").ap()
    # per-partition scalar columns: [128, item*8 + feat_tile]; rows 0..3 are
    # b1, ln_g, ln_b, b2
    bcols_d = nc.dram_tensor(
        "bcols", [4, P, BC * 8], f32, kind="ExternalInput"
    ).ap()
    WihT = nc.dram_tensor("WihT", [H + 1, G], f32r, kind="ExternalInput").ap()
    WhhT = nc.dram_tensor("WhhT", [HL, G], f32r, kind="ExternalInput").ap()
    WpT = nc.dram_tensor("WpT", [HL, E], f32r, kind="ExternalInput").ap()
    I8r = nc.dram_tensor("I8r", [BC, BC], f32r, kind="ExternalInput").ap()
    I8f = nc.dram_tensor("I8f", [BC, BC], f32, kind="ExternalInput").ap()
    if use_mask:
        msk = nc.dram_tensor("msk", [BC, S], f32, kind="ExternalInput").ap()
        mski = nc.dram_tensor(
            "mski", [BC, S], dt.int32, kind="ExternalInput"
        ).ap()
    partial = nc.dram_tensor(
        "partial", [TOK, E], f32, kind="ExternalOutput"
    ).ap()

    with tile.TileContext(nc) as tc:
        with (
            tc.tile_pool(name="persist", bufs=1) as persist,
            tc.tile_pool(name="scratch_dram", bufs=1, space="DRAM") as dpool,
        ):
            bcols = persist.tile([P, 4, BC * 8], f32)
            nc.sync.dma_start(
                out=bcols, in_=bcols_d.rearrange("s p c -> p s c")
            )
            i8r_sb = persist.tile([BC, BC], f32r)
            nc.sync.dma_start(out=i8r_sb, in_=I8r)
            i8f_sb = persist.tile([BC, BC], f32)
            nc.sync.dma_start(out=i8f_sb, in_=I8f)
            ones_r = persist.tile([1, P], f32r)
            nc.vector.memset(ones_r.bitcast(f32), 1.0)
            onescol = persist.tile([P, BC], f32r)
            nc.vector.memset(onescol.bitcast(f32), 1.0)
            eps_sb = persist.tile([P, 1], f32)
            nc.vector.memset(eps_sb, EPS)
            wp_sb = persist.tile([P, 4, E], f32r)
            nc.sync.dma_start(
                out=wp_sb, in_=WpT.rearrange("(k p) e -> p k e", p=P)
            )
            if use_mask:
                msk_sb = persist.tile([BC, S], f32)
                nc.sync.dma_start(out=msk_sb, in_=msk)
                mski_sb = persist.tile([BC, S], dt.int32)
                nc.sync.dma_start(out=mski_sb, in_=mski)

            zbuf = dpool.tile([S, BC, G], f32r)

            # ================= PHASE 1 =================
            if 1 not in phases:
                ph1 = False
            with (
                tc.tile_pool(name="p1wih", bufs=1) as p1wih,
                tc.tile_pool(name="p1w", bufs=10) as p1w,
                tc.tile_pool(name="p1x", bufs=2) as p1x,
                tc.tile_pool(name="p1a", bufs=2) as p1a,
                tc.tile_pool(name="p1r", bufs=2) as p1r,
                tc.tile_pool(name="psA", bufs=3, space="PSUM") as psA,
                tc.tile_pool(name="psS", bufs=2, space="PSUM") as psS,
            ):
                wih_sb = p1wih.tile([P, 8, G], f32r)
                nc.sync.dma_start(
                    out=wih_sb,
                    in_=WihT[:H, :].rearrange("(k p) g -> p k g", p=P),
                )
                bih_sb = p1wih.tile([1, G], f32r)
                nc.sync.dma_start(out=bih_sb, in_=WihT[H: H + 1, :])

                for i in range(BC if 1 in phases else 0):
                    xi = p1x.tile([P, 8, S], f32r, tag="xi")
                    nc.sync.dma_start(
                        out=xi,
                        in_=xT[:, i * S:(i + 1) * S].rearrange(
                            "(k p) t -> p k t", p=P
                        ),
                    )
                    # ---- h1T (+b1) ----
                    a0 = p1a.tile([P, 8, S], f32r, tag="a0")
                    for mh in range(2):
                        wb = [p1w.tile([P, 512], f32r, tag="w",
                                       name=f"w1b{i}_{mh}_{k}")
                              for k in range(8)]
                        for k in range(8):
                            nc.sync.dma_start(
                                out=wb[k],
                                in_=W1s[i, k * P:(k + 1) * P,
                                        mh * 512:(mh + 1) * 512],
                            )
                        for mm in range(4):
                            m = mh * 4 + mm
                            ps = psA.tile([P, S], f32, tag="mm")
                            for k in range(8):
                                nc.tensor.matmul(
                                    ps, wb[k][:, mm * P:(mm + 1) * P],
                                    xi[:, k, :],
                                    start=(k == 0), stop=(k == 7),
                                )
                            nc.scalar.activation(
                                out=a0[:, m, :], in_=ps, func=AF.Identity,
                                bias=bcols[:, 0, i * 8 + m: i * 8 + m + 1],
                            )

                    # ---- LN stats via ones-matmuls ----
                    a0sq = p1a.tile([P, 8, S], f32r, tag="a0sq", bufs=1)
                    nc.scalar.activation(
                        out=a0sq.rearrange("p m t -> p (m t)"),
                        in_=a0.rearrange("p m t -> p (m t)"),
                        func=AF.Square,
                    )
                    sps = psS.tile([BC, 2, S], f32, tag="sps")
                    for m in range(8):
                        nc.tensor.matmul(
                            sps[:, 0, :], onescol, a0[:, m, :],
                            start=(m == 0), stop=(m == 7),
                        )
                    for m in range(8):
                        nc.tensor.matmul(
                            sps[:, 1, :], onescol, a0sq[:, m, :],
                            start=(m == 0), stop=(m == 7),
                        )
                    # mr: [1, 2, S] f32r; slot0 = mean, slot1 = E[x^2]->rstd
                    mr = p1r.tile([1, 2, S], f32r, tag="mr")
                    nc.scalar.activation(
                        out=mr.rearrange("o s t -> o (s t)"),
                        in_=sps[0:1, :, :].rearrange("o s t -> o (s t)"),
                        func=AF.Identity, scale=1.0 / H,
                    )
                    msq = p1r.tile([1, S], f32, tag="msq")
                    nc.vector.tensor_mul(msq, mr[:, 0, :], mr[:, 0, :])
                    var = p1r.tile([1, S], f32, tag="var")
                    nc.vector.tensor_sub(var, mr[:, 1, :], msq)
                    sd = p1r.tile([1, S], f32, tag="sd")
                    nc.scalar.activation(out=sd, in_=var, func=AF.Sqrt,
                                         bias=eps_sb[0:1, :])
                    rtmp = p1r.tile([1, S], f32, tag="rtmp")
                    nc.vector.reciprocal(rtmp, sd)
                    nc.scalar.activation(out=mr[:, 1, :], in_=rtmp,
                                         func=AF.Identity)
                    # broadcast [1, 2S] -> [128, 2S] via ones-matmul
                    bps = psS.tile([P, 2, S], f32, tag="bps", bufs=1)
                    nc.tensor.matmul(
                        bps.rearrange("p s t -> p (s t)"), ones_r,
                        mr.rearrange("o s t -> o (s t)"),
                        start=True, stop=True,
                    )
                    mrB = p1r.tile([P, 2, S], f32, tag="mrB")
                    nc.scalar.activation(
                        out=mrB.rearrange("p s t -> p (s t)"),
                        in_=bps.rearrange("p s t -> p (s t)"),
                        func=AF.Identity,
                    )

                    # ---- LN apply + relu ----
                    a1 = p1a.tile([P, 8, S], f32r, tag="a1", bufs=1)
                    nc.vector.tensor_sub(
                        a1, a0,
                        mrB[:, 0:1, :].broadcast_to((P, 8, S)),
                    )
                    nc.vector.tensor_mul(
                        a1, a1,
                        mrB[:, 1:2, :].broadcast_to((P, 8, S)),
                    )
                    for m in range(8):
                        nc.vector.tensor_scalar(
                            out=a1[:, m, :], in0=a1[:, m, :],
                            scalar1=bcols[:, 1, i * 8 + m: i * 8 + m + 1],
                            scalar2=bcols[:, 2, i * 8 + m: i * 8 + m + 1],
                            op0=ALU.mult, op1=ALU.add,
                        )
                    nc.scalar.activation(
                        out=a1.rearrange("p m t -> p (m t)"),
                        in_=a1.rearrange("p m t -> p (m t)"),
                        func=AF.Relu,
                    )

                    # ---- h2T (+b2) ----
                    h2 = p1a.tile([P, 8, S], f32r, tag="h2")
                    for mh in range(2):
                        wb = [p1w.tile([P, 512], f32r, tag="w",
                                       name=f"w2b{i}_{mh}_{k}")
                              for k in range(8)]
                        for k in range(8):
                            nc.sync.dma_start(
                                out=wb[k],
                                in_=W2s[i, k * P:(k + 1) * P,
                                        mh * 512:(mh + 1) * 512],
                            )
                        for mm in range(4):
                            m = mh * 4 + mm
                            ps = psA.tile([P, S], f32, tag="mm")
                            for k in range(8):
                                nc.tensor.matmul(
                                    ps, wb[k][:, mm * P:(mm + 1) * P],
                                    a1[:, k, :],
                                    start=(k == 0), stop=(k == 7),
                                )
                            nc.scalar.activation(
                                out=h2[:, m, :], in_=ps, func=AF.Identity,
                                bias=bcols[:, 3, i * 8 + m: i * 8 + m + 1],
                            )

                    # ---- z = h2T.T @ WihT + b (token-major out) ----
                    for mc in range(2):
                        tsl = slice(mc * P, (mc + 1) * P)
                        for n in range(4):
                            gsl = slice(n * 512, (n + 1) * 512)
                            zp = psA.tile([P, 512], f32, tag="zmm", bufs=2)
                            for k in range(8):
                                nc.tensor.matmul(
                                    zp, h2[:, k, tsl],
                                    wih_sb[:, k, gsl],
                                    start=(k == 0), stop=False,
                                )
                            nc.tensor.matmul(
                                zp, ones_r, bih_sb[:, gsl],
                                start=False, stop=True,
                            )
                            zsb = p1x.tile([P, 512], f32r, tag="zsb", bufs=3)
                            nc.scalar.activation(
                                out=zsb, in_=zp, func=AF.Identity
                            )
                            nc.sync.dma_start(
                                out=zbuf[:, i, gsl][tsl, :],
                                in_=zsb,
                            )

            # ============ PHASE 2 + 3 scope (ysT lives here) ============
            with tc.tile_pool(name="pys", bufs=1) as pys:
                ysT = pys.tile([P, 4, TOK], f32r)

                with (
                    tc.tile_pool(name="p2whh", bufs=1) as p2whh,
                    tc.tile_pool(name="p2z", bufs=4) as p2z,
                    tc.tile_pool(name="p2s", bufs=1) as p2s,
                    tc.tile_pool(name="p2t", bufs=3) as p2t,
                    tc.tile_pool(name="psG", bufs=1, space="PSUM") as psG,
                    tc.tile_pool(name="psT", bufs=2, space="PSUM") as psT,
                ):
                    whh_sb = p2whh.tile([P, 4, G], f32r)
                    nc.sync.dma_start(
                        out=whh_sb,
                        in_=WhhT.rearrange("(k p) g -> p k g", p=P),
                    )
                    c_st = p2s.tile([BC, HL], f32)
                    nc.vector.memset(c_st, 0.0)
                    sig = p2s.tile([BC, G], f32)
                    hp = p2s.tile([BC, HL], f32)
                    if use_mask:
                        hstate = p2s.tile([BC, HL], f32)
                        nc.vector.memset(hstate, 0.0)
                        ym = p2s.tile([BC, HL], f32)
                    hT_prev = None

                    for s in range(nsteps if 2 in phases else 0):
                        zt = p2z.tile([BC, G], f32r, tag="zt")
                        nc.sync.dma_start(out=zt, in_=zbuf[s])
                        gp = psG.tile([BC, G], f32, tag="g")
                        for n in range(4):
                            gsl = slice(n * 512, (n + 1) * 512)
                            nc.tensor.matmul(
                                gp[:, gsl], i8r_sb, zt[:, gsl],
                                start=True, stop=(s == 0),
                            )
                            if s > 0:
                                if use_mask:
                                    lh = [hT_prev[:, k, :]
                                          for k in range(4)]
                                else:
                                    lh = [
                                        ysT[:, k, (s - 1) * BC: s * BC]
                                        for k in range(4)
                                    ]
                                for k in range(4):
                                    nc.tensor.matmul(
                                        gp[:, gsl], lh[k], whh_sb[:, k, gsl],
                                        start=False, stop=(k == 3),
                                    )
                            # elementwise for unit block n
                            bs = n * 512
                            usl = slice(n * P, (n + 1) * P)
                            nc.scalar.activation(
                                out=sig[:, bs: bs + 384],
                                in_=gp[:, bs: bs + 384], func=AF.Sigmoid,
                            )
                            nc.scalar.activation(
                                out=sig[:, bs + 384: bs + 512],
                                in_=gp[:, bs + 384: bs + 512], func=AF.Tanh,
                            )
                            tmp1 = p2t.tile([BC, P], f32, tag="t1")
                            tmp2 = p2t.tile([BC, P], f32, tag="t2")
                            nc.vector.tensor_mul(
                                tmp1, sig[:, bs + 128: bs + 256], c_st[:, usl]
                            )
                            nc.vector.tensor_mul(
                                tmp2, sig[:, bs: bs + 128],
                                sig[:, bs + 384: bs + 512],
                            )
                            if use_mask:
                                cnew = p2t.tile([BC, P], f32, tag="cn")
                                nc.vector.tensor_add(cnew, tmp1, tmp2)
                                nc.vector.copy_predicated(
                                    c_st[:, usl],
                                    mski_sb[:, s: s + 1].broadcast_to((BC, P)),
                                    cnew,
                                )
                            else:
                                nc.vector.tensor_add(c_st[:, usl], tmp1, tmp2)
                            tmp3 = p2t.tile([BC, P], f32, tag="t3")
                            nc.scalar.activation(
                                out=tmp3, in_=c_st[:, usl], func=AF.Tanh
                            )
                            nc.vector.tensor_mul(
                                hp[:, usl], sig[:, bs + 256: bs + 384], tmp3
                            )
                            if use_mask:
                                nc.vector.copy_predicated(
                                    hstate[:, usl],
                                    mski_sb[:, s: s + 1].broadcast_to((BC, P)),
                                    hp[:, usl],
                                )
                                nc.vector.tensor_scalar_mul(
                                    ym[:, usl], hstate[:, usl],
                                    msk_sb[:, s: s + 1],
                                )

                        # transpose h' into unit-major state / outputs
                        src = ym if use_mask else hp
                        tp = psT.tile([P, 4, BC], f32, tag="tp")
                        for n in range(4):
                            nc.tensor.transpose(
                                tp[:, n, :], src[:, n * P:(n + 1) * P], i8f_sb
                            )
                        nc.vector.tensor_copy(
                            ysT[:, :, s * BC:(s + 1) * BC], tp
                        )
                        if use_mask:
                            tp2 = psT.tile([P, 4, BC], f32, tag="tp")
                            for n in range(4):
                                nc.tensor.transpose(
                                    tp2[:, n, :],
                                    hstate[:, n * P:(n + 1) * P], i8f_sb,
                                )
                            hT_prev = p2t.tile([P, 4, BC], f32r, tag="hT")
                            nc.vector.tensor_copy(hT_prev, tp2)

                # ================= PHASE 3 =================
                with (
                    tc.tile_pool(name="p3", bufs=4) as p3,
                    tc.tile_pool(name="psP", bufs=4, space="PSUM") as psP,
                ):
                    for mt in range(TOK // P if 3 in phases else 0):
                        pp = psP.tile([P, E], f32, tag="pp")
                        for k in range(4):
                            nc.tensor.matmul(
                                pp,
                                ysT[:, k, mt * P:(mt + 1) * P],
                                wp_sb[:, k, :],
                                start=(k == 0), stop=(k == 3),
                            )
                        ot = p3.tile([P, E], f32, tag="ot")
                        nc.scalar.activation(out=ot, in_=pp, func=AF.Identity)
                        nc.sync.dma_start(
                            out=partial[mt * P:(mt + 1) * P, :], in_=ot
                        )

    nc.finalize()
    return nc


def _prep_core_inputs(core, perm, seq, am, li, W1, b1, ln_g, ln_b, W2, b2,
                      Wih, Whh, bvec, Wp, use_mask):
    q = core % 4
    bwd = core >= 4
    items = perm[q * BC:(q + 1) * BC]
    gperm = _gate_perm()

    x = seq[items]                          # [8, S, H]
    if bwd:
        x = x[:, ::-1, :]
    xT = np.ascontiguousarray(
        x.transpose(2, 0, 1).reshape(H, TOK), dtype=np.float32
    )
    langs = li[items]
    W1s = np.ascontiguousarray(W1[langs], dtype=np.float32)
    W2s = np.ascontiguousarray(W2[langs], dtype=np.float32)

    def cols(v):                            # [L,1024] -> [128, item*8+m]
        vv = v[langs]
        return vv.reshape(BC, 8, P).transpose(2, 0, 1).reshape(P, BC * 8)

    bcols = np.ascontiguousarray(
        np.stack([cols(b1), cols(ln_g), cols(ln_b), cols(b2)], axis=0),
        dtype=np.float32,
    )

    WihT = np.empty((H + 1, G), dtype=np.float32)
    WihT[:H, :] = Wih.T[:, gperm]
    WihT[H, :] = bvec[gperm]
    WhhT = np.ascontiguousarray(Whh.T[:, gperm], dtype=np.float32)
    d0 = HL if bwd else 0
    WpT = np.ascontiguousarray(Wp[:, d0:d0 + HL].T, dtype=np.float32)

    m = {
        "xT": xT, "W1s": W1s, "W2s": W2s, "bcols": bcols,
        "WihT": WihT, "WhhT": WhhT, "WpT": WpT,
        "I8r": np.eye(BC, dtype=np.float32),
        "I8f": np.eye(BC, dtype=np.float32),
    }
    if use_mask:
        mm = am[items].astype(np.float32)
        if bwd:
            mm = mm[:, ::-1]
        m["msk"] = np.ascontiguousarray(mm)
        m["mski"] = np.ascontiguousarray(mm.astype(np.int32))
    return m


def kernel(sequence_output, attention_mask, language_ids, W1, b1, ln_g, ln_b,
           W2, b2, Wih_f, Whh_f, b_f, Wih_b, Whh_b, b_b, Wp, bp):
    from concourse.bass_utils import run_bass_kernel_spmd

    seq = np.asarray(sequence_output, dtype=np.float32)
    am = np.asarray(attention_mask)
    li = np.asarray(language_ids).astype(np.int64)
    use_mask = not bool(np.all(am == 1))

    key = ("nc", use_mask)
    if key not in _CACHE:
        _CACHE[key] = _build_nc(use_mask)
    nc = _CACHE[key]

    perm = np.argsort(li, kind="stable")
    in_maps = []
    for core in range(NCORES):
        bwd = core >= 4
        in_maps.append(
            _prep_core_inputs(
                core, perm, seq, am, li,
                np.asarray(W1, np.float32), np.asarray(b1, np.float32),
                np.asarray(ln_g, np.float32), np.asarray(ln_b, np.float32),
                np.asarray(W2, np.float32), np.asarray(b2, np.float32),
                np.asarray(Wih_b if bwd else Wih_f, np.float32),
                np.asarray(Whh_b if bwd else Whh_f, np.float32),
                np.asarray(b_b if bwd else b_f, np.float32),
                np.asarray(Wp, np.float32), use_mask,
            )
        )

    trace = bool(os.environ.get("KERNEL_TRACE"))
    res = run_bass_kernel_spmd(
        nc, in_maps, core_ids=list(range(NCORES)), trace=trace
    )
    LAST_RUN["exec_time_ns"] = res.exec_time_ns
    LAST_RUN["profile_json"] = res.profile_json
    # partial rows are ordered (t, b_local)
    outs = [
        r["partial"].reshape(S, BC, E).transpose(1, 0, 2) for r in res.results
    ]

    out = np.empty((B, S, E), dtype=np.float32)
    bp32 = np.asarray(bp, dtype=np.float32)
    for q in range(4):
        items = perm[q * BC:(q + 1) * BC]
        pf = outs[q]                        # [8, S, E]
        pb = outs[q + 4][:, ::-1, :]        # un-reverse time
        out[items] = pf + pb + bp32
    return out



# revision 2
# speedup vs baseline: 1.0134x; 1.0134x over previous
"""Trainium2 Bass kernel v2 for nn_EntityEncoder (adapters + BiLSTM + proj).

Sharding: 8 cores = 4 batch-quarters x 2 LSTM directions (as v1).

Key changes vs v1:
  - fp16 matmul operands everywhere (1 cyc/col vs fp32r ~3).
  - Phase 2 is weights-stationary: gates land TRANSPOSED [units, batch]
    in PSUM, so elementwise uses all 128 lanes and h' needs no per-step
    PE transpose (its layout is already next step's moving operand).
  - Masking is folded into the gate pre-activations: phase 1 adds
    30*(m-1) to z via a K=2 matmul row, so sigmoid/tanh saturate to
    0/-1 on masked steps. Equivalent to reference retention semantics
    because masks are monotone (fwd: suffix masked; bwd: prefix masked).
  - z kept resident in SBUF as fp16; injected into PSUM via one
    identity matmul per step (no phase-2 DMA at all).

Gate chunk order (unit-chunks of 128 on the partition axis):
  chunks 0-3 = g, 4-7 = i, 8-11 = f, 12-15 = o
so tanh(g) can start earliest and sigma(i,f) = one [128,64]-wide
activation; sigma(o) is last and only feeds the final h-mul.
"""

import os

import numpy as np

B, S, H, HL, E, L = 32, 256, 1024, 512, 256, 5
G = 4 * HL            # 2048 gate width
NCORES = 8
BC = 8                # batch items per core
TOK = BC * S          # tokens per core
EPS = 1e-5
P = 128
NEG = 30.0            # mask kill bias

_CACHE = {}
LAST_RUN = {}

# chunk order on the gate axis: [g, f, i, o] x 4 unit-subchunks
_GATE_OF_CHUNK = [2, 2, 2, 2, 1, 1, 1, 1, 0, 0, 0, 0, 3, 3, 3, 3]


def _chunk_perm():
    """perm[c*128+p] = original gate index for chunk c, unit p.
    Torch gate order in weights: i(0) f(1) g(2) o(3)."""
    perm = np.zeros(G, dtype=np.int64)
    for c in range(16):
        gate = _GATE_OF_CHUNK[c]
        sub = [0, 1, 2, 3][c % 4]
        u = np.arange(128) + sub * 128
        perm[c * 128:(c + 1) * 128] = gate * HL + u
    return perm


def _build_nc(nsteps=S, phases=(1, 2, 3)):
    import concourse.tile as tile
    import concourse.mybir as mybir
    from concourse import bacc

    dt = mybir.dt
    f32 = dt.float32
    f16 = dt.float16
    AF = mybir.ActivationFunctionType
    ALU = mybir.AluOpType

    nc = bacc.Bacc(
        "TRN2", target_bir_lowering=False, debug=False, num_devices=NCORES
    )

    # ---------------- I/O ----------------
    xT = nc.dram_tensor("xT", [H, TOK], f16, kind="ExternalInput").ap()
    W1s = nc.dram_tensor("W1s", [BC, H, H], f16, kind="ExternalInput").ap()
    W2s = nc.dram_tensor("W2s", [BC, H, H], f16, kind="ExternalInput").ap()
    # rows 0..3 are b1, ln_g, ln_b, b2; col = item*8 + feat_chunk
    bcols_d = nc.dram_tensor(
        "bcols", [4, P, BC * 8], f32, kind="ExternalInput"
    ).ap()
    # Wih stationary tiles: [k, c, feat128, unit128] (lhsT per tile)
    WihS = nc.dram_tensor("WihS", [8, 16, P, P], f16, kind="ExternalInput").ap()
    # bias+mask: stationary [2, G] rows (b, NEG*ones); moving [2, TOK]
    # rows (ones, m-1)
    biasS = nc.dram_tensor("biasS", [2, G], f16, kind="ExternalInput").ap()
    mrow = nc.dram_tensor("mrow", [2, TOK], f16, kind="ExternalInput").ap()
    WhhS = nc.dram_tensor("WhhS", [4, 16, P, P], f16, kind="ExternalInput").ap()
    WpT = nc.dram_tensor("WpT", [P, 4, E], f16, kind="ExternalInput").ap()
    I128 = nc.dram_tensor("I128", [P, P], f16, kind="ExternalInput").ap()
    OnesP = nc.dram_tensor("OnesP", [P, P], f16, kind="ExternalInput").ap()
    partial = nc.dram_tensor(
        "partial", [TOK, E], f32, kind="ExternalOutput"
    ).ap()

    with tile.TileContext(nc) as tc:
        with tc.tile_pool(name="persist", bufs=1) as persist:
            bcols = persist.tile([P, 4, BC * 8], f32)
            nc.sync.dma_start(out=bcols, in_=bcols_d.rearrange("s p c -> p s c"))
            i128_sb = persist.tile([P, P], f16)
            nc.sync.dma_start(out=i128_sb, in_=I128)
            onesp = persist.tile([P, P], f16)
            nc.sync.dma_start(out=onesp, in_=OnesP)
            eps_sb = persist.tile([P, 1], f32)
            nc.vector.memset(eps_sb, EPS)

            # z resident in SBUF: [128, chunk, token] fp16
            zT = persist.tile([P, 16, TOK], f16)
            # lstm hidden history, unit-major: [128, k, token] fp16
            ysT = persist.tile([P, 4, TOK], f16)

            # ================= PHASE 1 =================
            with (
                tc.tile_pool(name="p1wih", bufs=1) as p1wih,
                tc.tile_pool(name="p1w", bufs=8) as p1w,
                tc.tile_pool(name="p1misc", bufs=1) as p1misc,
                tc.tile_pool(name="p1x", bufs=2) as p1x,
                tc.tile_pool(name="p1a", bufs=2) as p1a,
                tc.tile_pool(name="p1h2", bufs=1) as p1h2,
                tc.tile_pool(name="p1r", bufs=2) as p1r,
                tc.tile_pool(name="psA", bufs=3, space="PSUM") as psA,
                tc.tile_pool(name="psS", bufs=2, space="PSUM") as psS,
                tc.tile_pool(name="psZ", bufs=2, space="PSUM") as psZ,
            ):
                # Wih stationary tiles in SBUF: [128, k, c, 128]
                # (DMA emitted later, at i==1, so it doesn't block the
                # first items' xi/wb loads in the DMA queues)
                wih_sb = p1wih.tile([P, 8, 16, P], f16)

                mrow_sb = p1misc.tile([2, TOK], f16)
                nc.sync.dma_start(out=mrow_sb, in_=mrow)
                biasS_sb = p1misc.tile([2, G], f16)
                nc.sync.dma_start(out=biasS_sb, in_=biasS)
                # h2 quad buffers: [128, featchunk, item-in-quad, S]
                h2q = [
                    p1h2.tile([P, 8, 4, S], f16, name=f"h2q{q}")
                    for q in range(2)
                ]

                def emit_h1(i):
                    """xi DMA + h1 matmuls + inline Square/stat-sums."""
                    xi = p1x.tile([P, 8, S], f16, tag="xi", name=f"xi{i}")
                    nc.sync.dma_start(
                        out=xi,
                        in_=xT[:, i * S:(i + 1) * S].rearrange(
                            "(k p) t -> p k t", p=P
                        ),
                    )
                    a0 = p1a.tile([P, 8, S], f16, tag="a0", name=f"a0_{i}")
                    sps0 = psS.tile([P, S], f32, tag="sps0", bufs=1,
                                    name=f"sps0_{i}")
                    sps1 = psS.tile([P, S], f32, tag="sps1", bufs=1,
                                    name=f"sps1_{i}")
                    for mh in range(2):
                        wb = [p1w.tile([P, 512], f16, tag="w",
                                       name=f"w1b{i}_{mh}_{k}")
                              for k in range(8)]
                        for k in range(8):
                            nc.sync.dma_start(
                                out=wb[k],
                                in_=W1s[i, k * P:(k + 1) * P,
                                        mh * 512:(mh + 1) * 512],
                            )
                        for mm in range(4):
                            m = mh * 4 + mm
                            ps = psA.tile([P, S], f32, tag="mm",
                                          name=f"ps1_{i}_{m}")
                            for k in range(8):
                                nc.tensor.matmul(
                                    ps, wb[k][:, mm * P:(mm + 1) * P],
                                    xi[:, k, :],
                                    start=(k == 0), stop=(k == 7),
                                )
                            nc.scalar.activation(
                                out=a0[:, m, :], in_=ps, func=AF.Identity,
                                bias=bcols[:, 0, i * 8 + m: i * 8 + m + 1],
                            )
                            sq = p1a.tile([P, S], f16, tag="sq",
                                          name=f"sq{i}_{m}")
                            nc.scalar.activation(
                                out=sq, in_=a0[:, m, :], func=AF.Square,
                            )
                            nc.tensor.matmul(
                                sps0, onesp, a0[:, m, :],
                                start=(m == 0), stop=(m == 7),
                                skip_group_check=True,
                            )
                            nc.tensor.matmul(
                                sps1, onesp, sq,
                                start=(m == 0), stop=(m == 7),
                                skip_group_check=True,
                            )
                    mrB = p1r.tile([P, 2, S], f32, tag="mrB",
                                   name=f"mrB{i}")
                    nc.scalar.activation(
                        out=mrB[:, 0, :], in_=sps0,
                        func=AF.Identity, scale=1.0 / H,
                    )
                    nc.scalar.activation(
                        out=mrB[:, 1, :], in_=sps1,
                        func=AF.Identity, scale=1.0 / H,
                    )
                    scr = p1r.tile([P, S], f32, tag="scr", name=f"scr{i}")
                    nc.vector.tensor_mul(scr, mrB[:, 0, :], mrB[:, 0, :])
                    nc.vector.tensor_sub(scr, mrB[:, 1, :], scr)
                    # rstd = 1/sqrt(|var| + eps); var >= 0 so same as
                    # rsqrt, and this func shares its act table with
                    # identity/square/relu (no ACT_TABLE_LOAD swaps)
                    nc.scalar.activation(out=mrB[:, 1, :], in_=scr,
                                         func=AF.Abs_reciprocal_sqrt,
                                         bias=eps_sb)
                    return a0, mrB

                def emit_rest(i, a0, mrB):
                    """LN apply + h2 for item i."""
                    a1 = p1a.tile([P, 8, S], f16, tag="a1", name=f"a1_{i}")
                    for m in range(8):
                        nc.vector.tensor_sub(
                            a1[:, m, :], a0[:, m, :], mrB[:, 0, :]
                        )
                        nc.vector.tensor_mul(
                            a1[:, m, :], a1[:, m, :], mrB[:, 1, :]
                        )
                        nc.vector.tensor_scalar(
                            out=a1[:, m, :], in0=a1[:, m, :],
                            scalar1=bcols[:, 1, i * 8 + m: i * 8 + m + 1],
                            scalar2=bcols[:, 2, i * 8 + m: i * 8 + m + 1],
                            op0=ALU.mult, op1=ALU.add,
                        )
                        nc.scalar.activation(
                            out=a1[:, m, :], in_=a1[:, m, :], func=AF.Relu,
                        )

                    q, iq = i // 4, i % 4
                    for mh in range(2):
                        wb = [p1w.tile([P, 512], f16, tag="w",
                                       name=f"w2b{i}_{mh}_{k}")
                              for k in range(8)]
                        for k in range(8):
                            nc.sync.dma_start(
                                out=wb[k],
                                in_=W2s[i, k * P:(k + 1) * P,
                                        mh * 512:(mh + 1) * 512],
                            )
                        for mm in range(4):
                            m = mh * 4 + mm
                            ps = psA.tile([P, S], f32, tag="mm",
                                          name=f"ps2_{i}_{m}")
                            for k in range(8):
                                nc.tensor.matmul(
                                    ps, wb[k][:, mm * P:(mm + 1) * P],
                                    a1[:, k, :],
                                    start=(k == 0), stop=(k == 7),
                                )
                            nc.scalar.activation(
                                out=h2q[q][:, m, iq, :], in_=ps,
                                func=AF.Identity,
                                bias=bcols[:, 3, i * 8 + m: i * 8 + m + 1],
                            )

                def emit_z(q):
                    for c in range(16):
                        for th in range(2):  # 512-token halves of quad
                            zp = psZ.tile([P, 512], f32, tag="zp",
                                          name=f"zp{q}_{c}_{th}")
                            rhs = h2q[q].rearrange("p m i t -> p m (i t)")
                            for k in range(8):
                                nc.tensor.matmul(
                                    zp, wih_sb[:, k, c, :],
                                    rhs[:, k, th * 512:(th + 1) * 512],
                                    start=(k == 0), stop=False,
                                )
                            tsl = slice(q * 1024 + th * 512,
                                        q * 1024 + (th + 1) * 512)
                            nc.tensor.matmul(
                                zp, biasS_sb[:, c * P:(c + 1) * P],
                                mrow_sb[:, tsl],
                                start=False, stop=True,
                            )
                            nc.scalar.activation(
                                out=zT[:, c, tsl], in_=zp,
                                func=AF.Identity,
                            )

                # software pipeline: h1 of item i+1 is emitted before the
                # LN/h2 of item i so the PE FIFO never drains during the
                # LN latency chain
                if 1 in phases:
                    pending = emit_h1(0)
                    for i in range(BC):
                        nxt = emit_h1(i + 1) if i + 1 < BC else None
                        if i == 1:
                            nc.sync.dma_start(
                                out=wih_sb,
                                in_=WihS.rearrange("k c p u -> p k c u"),
                            )
                        emit_rest(i, *pending)
                        pending = nxt
                        if i % 4 == 3:
                            emit_z(i // 4)

            # ================= PHASE 2 =================
            with (
                tc.tile_pool(name="p2whh", bufs=1) as p2whh,
                tc.tile_pool(name="p2s", bufs=1) as p2s,
                tc.tile_pool(name="p2t", bufs=2) as p2t,
                tc.tile_pool(name="psG", bufs=2, space="PSUM") as psG,
            ):
                whh_sb = p2whh.tile([P, 4, 16, P], f16)
                nc.sync.dma_start(
                    out=whh_sb, in_=WhhS.rearrange("k c p u -> p k c u")
                )
                c_st = p2s.tile([P, 4, BC], f32)
                nc.vector.memset(c_st.rearrange("p a b -> p (a b)"), 0.0)
                # zT tokens are item-major (i*S + t); view for per-step
                # strided reads of all 8 items at time s
                zT4 = zT.rearrange("p c (i t) -> p c i t", i=BC)

                for s in range(nsteps if 2 in phases else 0):
                    # one PSUM tile per gate -> independent dep tracking,
                    # so each gate's activation fires right after its own
                    # accumulation group stops (not after the whole burst)
                    gps = [
                        psG.tile([P, 4, BC], f32, tag=f"g{g}",
                                 name=f"gp{g}_{s}")
                        for g in range(4)
                    ]
                    bsl = slice(s * BC, (s + 1) * BC)
                    psl = slice((s - 1) * BC, s * BC)
                    # gate order: 0 g, 1 f, 2 i, 3 o (chunks 4g..4g+3)
                    for g in range(4):
                        csl = slice(4 * g, 4 * g + 4)
                        nc.tensor.matmul(
                            gps[g].rearrange("p c b -> p (c b)"),
                            i128_sb,
                            zT4[:, csl, :, s].rearrange("p c i -> p (c i)"),
                            start=True, stop=(s == 0),
                        )
                        if s > 0:
                            for cc in range(4):
                                c = 4 * g + cc
                                for k in range(4):
                                    nc.tensor.matmul(
                                        gps[g][:, cc, :], whh_sb[:, k, c, :],
                                        ysT[:, k, psl],
                                        start=False,
                                        stop=(cc == 3 and k == 3),
                                        skip_group_check=True,
                                    )
                    sig = p2t.tile([P, 16, BC], f32, tag="sig")
                    nc.scalar.activation(
                        out=sig[:, 0:4, :].rearrange("p c b -> p (c b)"),
                        in_=gps[0].rearrange("p c b -> p (c b)"),
                        func=AF.Tanh,
                    )
                    nc.scalar.activation(
                        out=sig[:, 4:8, :].rearrange("p c b -> p (c b)"),
                        in_=gps[1].rearrange("p c b -> p (c b)"),
                        func=AF.Sigmoid,
                    )
                    nc.scalar.activation(
                        out=sig[:, 8:12, :].rearrange("p c b -> p (c b)"),
                        in_=gps[2].rearrange("p c b -> p (c b)"),
                        func=AF.Sigmoid,
                    )
                    nc.scalar.activation(
                        out=sig[:, 12:16, :].rearrange("p c b -> p (c b)"),
                        in_=gps[3].rearrange("p c b -> p (c b)"),
                        func=AF.Sigmoid,
                    )
                    # c/h chain split in unit-halves so ysT[:, 0:2] is
                    # released before the second half computes
                    for hh in range(2):
                        u = slice(2 * hh, 2 * hh + 2)
                        uo = slice(12 + 2 * hh, 14 + 2 * hh)
                        uf = slice(4 + 2 * hh, 6 + 2 * hh)
                        ui = slice(8 + 2 * hh, 10 + 2 * hh)
                        t1 = p2t.tile([P, 2, BC], f32, tag=f"t1{hh}")
                        nc.vector.tensor_mul(t1, sig[:, uf, :], c_st[:, u, :])
                        t2 = p2t.tile([P, 2, BC], f32, tag=f"t2{hh}")
                        nc.vector.tensor_mul(t2, sig[:, ui, :], sig[:, u, :])
                        nc.vector.tensor_add(c_st[:, u, :], t1, t2)
                        tc3 = p2t.tile([P, 2, BC], f32, tag=f"tc{hh}")
                        nc.scalar.activation(
                            out=tc3.rearrange("p a b -> p (a b)"),
                            in_=c_st[:, u, :].rearrange("p a b -> p (a b)"),
                            func=AF.Tanh,
                        )
                        nc.vector.tensor_mul(
                            ysT[:, u, bsl], sig[:, uo, :], tc3,
                        )

            # ================= PHASE 3 =================
            with (
                tc.tile_pool(name="p3", bufs=4) as p3,
                tc.tile_pool(name="p3w", bufs=1) as p3w,
                tc.tile_pool(name="psP", bufs=4, space="PSUM") as psP,
            ):
                wp_sb = p3w.tile([P, 4, E], f16)
                nc.sync.dma_start(out=wp_sb, in_=WpT)
                for mt in range(TOK // P if 3 in phases else 0):
                    pp = psP.tile([P, E], f32, tag="pp")
                    for k in range(4):
                        nc.tensor.matmul(
                            pp, ysT[:, k, mt * P:(mt + 1) * P],
                            wp_sb[:, k, :],
                            start=(k == 0), stop=(k == 3),
                        )
                    ot = p3.tile([P, E], f32, tag="ot")
                    nc.scalar.activation(out=ot, in_=pp, func=AF.Identity)
                    nc.sync.dma_start(
                        out=partial[mt * P:(mt + 1) * P, :], in_=ot
                    )

    nc.finalize()
    return nc


def _prep_core_inputs(core, perm, seq, am, li, W1, b1, ln_g, ln_b, W2, b2,
                      Wih, Whh, bvec, Wp):
    q = core % 4
    bwd = core >= 4
    items = perm[q * BC:(q + 1) * BC]
    cperm = _chunk_perm()

    x = seq[items]                          # [8, S, H]
    mm = am[items].astype(np.float32)       # [8, S]
    if bwd:
        x = x[:, ::-1, :]
        mm = mm[:, ::-1]
    xT = np.ascontiguousarray(
        x.transpose(2, 0, 1).reshape(H, TOK), dtype=np.float16
    )
    langs = li[items]
    W1s = np.ascontiguousarray(W1[langs], dtype=np.float16)
    W2s = np.ascontiguousarray(W2[langs], dtype=np.float16)

    def cols(v):                            # [L,1024] -> [128, item*8+m]
        vv = v[langs]
        return vv.reshape(BC, 8, P).transpose(2, 0, 1).reshape(P, BC * 8)

    bcols = np.ascontiguousarray(
        np.stack([cols(b1), cols(ln_g), cols(ln_b), cols(b2)], axis=0),
        dtype=np.float32,
    )

    # Wih stationary tiles [k, c, feat128, unit128]: lhsT = Wih.T chunk
    WihP = Wih[cperm, :]                    # [G, H] permuted gate rows
    WihS = np.ascontiguousarray(
        WihP.reshape(16, P, 8, P).transpose(2, 0, 3, 1), dtype=np.float16
    )  # [k, c, feat, unit]
    biasS = np.empty((2, G), dtype=np.float16)
    biasS[0] = bvec[cperm]
    biasS[1] = NEG
    mrow = np.empty((2, TOK), dtype=np.float16)
    mrow[0] = 1.0
    mrow[1] = (mm - 1.0).reshape(TOK)

    WhhP = Whh[cperm, :]                    # [G, HL]
    WhhS = np.ascontiguousarray(
        WhhP.reshape(16, P, 4, P).transpose(2, 0, 3, 1), dtype=np.float16
    )  # [k, c, feat, unit]

    d0 = HL if bwd else 0
    WpT = np.ascontiguousarray(
        Wp[:, d0:d0 + HL].T.reshape(4, P, E).transpose(1, 0, 2),
        dtype=np.float16,
    )  # [p, k, e]

    return {
        "xT": xT, "W1s": W1s, "W2s": W2s, "bcols": bcols,
        "WihS": WihS, "biasS": biasS, "mrow": mrow, "WhhS": WhhS,
        "WpT": WpT, "I128": np.eye(P, dtype=np.float16),
        "OnesP": np.ones((P, P), dtype=np.float16),
    }


def kernel(sequence_output, attention_mask, language_ids, W1, b1, ln_g, ln_b,
           W2, b2, Wih_f, Whh_f, b_f, Wih_b, Whh_b, b_b, Wp, bp):
    from concourse.bass_utils import run_bass_kernel_spmd

    seq = np.asarray(sequence_output, dtype=np.float32)
    am = np.asarray(attention_mask)
    li = np.asarray(language_ids).astype(np.int64)

    key = "nc2"
    if key not in _CACHE:
        _CACHE[key] = _build_nc()
    nc = _CACHE[key]

    perm = np.argsort(li, kind="stable")
    in_maps = []
    for core in range(NCORES):
        bwd = core >= 4
        in_maps.append(
            _prep_core_inputs(
                core, perm, seq, am, li,
                np.asarray(W1, np.float32), np.asarray(b1, np.float32),
                np.asarray(ln_g, np.float32), np.asarray(ln_b, np.float32),
                np.asarray(W2, np.float32), np.asarray(b2, np.float32),
                np.asarray(Wih_b if bwd else Wih_f, np.float32),
                np.asarray(Whh_b if bwd else Whh_f, np.float32),
                np.asarray(b_b if bwd else b_f, np.float32),
                np.asarray(Wp, np.float32),
            )
        )

    trace = bool(os.environ.get("KERNEL_TRACE"))
    res = run_bass_kernel_spmd(
        nc, in_maps, core_ids=list(range(NCORES)), trace=trace
    )
    LAST_RUN["exec_time_ns"] = res.exec_time_ns
    LAST_RUN["profile_json"] = res.profile_json
    # partial rows are ordered (t, b_local): ysT tokens are step-major
    outs = [
        r["partial"].reshape(S, BC, E).transpose(1, 0, 2) for r in res.results
    ]

    out = np.empty((B, S, E), dtype=np.float32)
    bp32 = np.asarray(bp, dtype=np.float32)
    for q in range(4):
        items = perm[q * BC:(q + 1) * BC]
        pf = outs[q]                        # [8, S, E]
        pb = outs[q + 4][:, ::-1, :]        # un-reverse time
        out[items] = pf + pb + bp32
    return out


# revision 3
# speedup vs baseline: 1.0151x; 1.0017x over previous
"""Trainium2 Bass kernel v2 for nn_EntityEncoder (adapters + BiLSTM + proj).

Sharding: 8 cores = 4 batch-quarters x 2 LSTM directions (as v1).

Key changes vs v1:
  - fp16 matmul operands everywhere (1 cyc/col vs fp32r ~3).
  - Phase 2 is weights-stationary: gates land TRANSPOSED [units, batch]
    in PSUM, so elementwise uses all 128 lanes and h' needs no per-step
    PE transpose (its layout is already next step's moving operand).
  - Masking is folded into the gate pre-activations: phase 1 adds
    30*(m-1) to z via a K=2 matmul row, so sigmoid/tanh saturate to
    0/-1 on masked steps. Equivalent to reference retention semantics
    because masks are monotone (fwd: suffix masked; bwd: prefix masked).
  - z kept resident in SBUF as fp16; injected into PSUM via one
    identity matmul per step (no phase-2 DMA at all).

Gate chunk order (unit-chunks of 128 on the partition axis):
  chunks 0-3 = g, 4-7 = i, 8-11 = f, 12-15 = o
so tanh(g) can start earliest and sigma(i,f) = one [128,64]-wide
activation; sigma(o) is last and only feeds the final h-mul.
"""

import os

import numpy as np

B, S, H, HL, E, L = 32, 256, 1024, 512, 256, 5
G = 4 * HL            # 2048 gate width
NCORES = 8
BC = 8                # batch items per core
TOK = BC * S          # tokens per core
EPS = 1e-5
P = 128
NEG = 30.0            # mask kill bias

_CACHE = {}
LAST_RUN = {}

# chunk order on the gate axis: [g, f, i, o] x 4 unit-subchunks
_GATE_OF_CHUNK = [2, 2, 2, 2, 1, 1, 1, 1, 0, 0, 0, 0, 3, 3, 3, 3]


def _chunk_perm():
    """perm[c*128+p] = original gate index for chunk c, unit p.
    Torch gate order in weights: i(0) f(1) g(2) o(3)."""
    perm = np.zeros(G, dtype=np.int64)
    for c in range(16):
        gate = _GATE_OF_CHUNK[c]
        sub = [0, 1, 2, 3][c % 4]
        u = np.arange(128) + sub * 128
        perm[c * 128:(c + 1) * 128] = gate * HL + u
    return perm


def _build_nc(nsteps=S, phases=(1, 2, 3)):
    import concourse.tile as tile
    import concourse.mybir as mybir
    from concourse import bacc

    dt = mybir.dt
    f32 = dt.float32
    f16 = dt.float16
    AF = mybir.ActivationFunctionType
    ALU = mybir.AluOpType

    nc = bacc.Bacc(
        "TRN2", target_bir_lowering=False, debug=False, num_devices=NCORES
    )

    # ---------------- I/O ----------------
    xT = nc.dram_tensor("xT", [H, TOK], f16, kind="ExternalInput").ap()
    W1s = nc.dram_tensor("W1s", [BC, H, H], f16, kind="ExternalInput").ap()
    W2s = nc.dram_tensor("W2s", [BC, H, H], f16, kind="ExternalInput").ap()
    # rows 0..3 are b1, ln_g, ln_b, b2; col = item*8 + feat_chunk
    bcols_d = nc.dram_tensor(
        "bcols", [4, P, BC * 8], f32, kind="ExternalInput"
    ).ap()
    # Wih stationary tiles: [k, c, feat128, unit128] (lhsT per tile)
    WihS = nc.dram_tensor("WihS", [8, 16, P, P], f16, kind="ExternalInput").ap()
    # bias+mask: stationary [2, G] rows (b, NEG*ones); moving [2, TOK]
    # rows (ones, m-1)
    biasS = nc.dram_tensor("biasS", [2, G], f16, kind="ExternalInput").ap()
    mrow = nc.dram_tensor("mrow", [2, TOK], f16, kind="ExternalInput").ap()
    WhhS = nc.dram_tensor("WhhS", [4, 16, P, P], f16, kind="ExternalInput").ap()
    WpT = nc.dram_tensor("WpT", [P, 4, E], f16, kind="ExternalInput").ap()
    I128 = nc.dram_tensor("I128", [P, P], f16, kind="ExternalInput").ap()
    OnesP = nc.dram_tensor("OnesP", [P, P], f16, kind="ExternalInput").ap()
    partial = nc.dram_tensor(
        "partial", [TOK, E], f32, kind="ExternalOutput"
    ).ap()

    with tile.TileContext(nc) as tc:
        with tc.tile_pool(name="persist", bufs=1) as persist:
            bcols = persist.tile([P, 4, BC * 8], f32)
            nc.sync.dma_start(out=bcols, in_=bcols_d.rearrange("s p c -> p s c"))
            i128_sb = persist.tile([P, P], f16)
            nc.sync.dma_start(out=i128_sb, in_=I128)
            onesp = persist.tile([P, P], f16)
            nc.sync.dma_start(out=onesp, in_=OnesP)
            eps_sb = persist.tile([P, 1], f32)
            nc.vector.memset(eps_sb, EPS)

            # z resident in SBUF: [128, chunk, token] fp16
            zT = persist.tile([P, 16, TOK], f16)
            # lstm hidden history, unit-major: [128, k, token] fp16
            ysT = persist.tile([P, 4, TOK], f16)

            # ================= PHASE 1 =================
            with (
                tc.tile_pool(name="p1wih", bufs=1) as p1wih,
                tc.tile_pool(name="p1w", bufs=8) as p1w,
                tc.tile_pool(name="p1misc", bufs=1) as p1misc,
                tc.tile_pool(name="p1x", bufs=2) as p1x,
                tc.tile_pool(name="p1a", bufs=2) as p1a,
                tc.tile_pool(name="p1h2", bufs=1) as p1h2,
                tc.tile_pool(name="p1r", bufs=2) as p1r,
                tc.tile_pool(name="psA", bufs=3, space="PSUM") as psA,
                tc.tile_pool(name="psS", bufs=2, space="PSUM") as psS,
                tc.tile_pool(name="psZ", bufs=2, space="PSUM") as psZ,
            ):
                # Wih stationary tiles in SBUF: [128, k, c, 128]
                # (DMA emitted later, at i==1, so it doesn't block the
                # first items' xi/wb loads in the DMA queues)
                wih_sb = p1wih.tile([P, 8, 16, P], f16)

                mrow_sb = p1misc.tile([2, TOK], f16)
                nc.sync.dma_start(out=mrow_sb, in_=mrow)
                biasS_sb = p1misc.tile([2, G], f16)
                nc.sync.dma_start(out=biasS_sb, in_=biasS)
                # h2 quad buffers: [128, featchunk, item-in-quad, S]
                h2q = [
                    p1h2.tile([P, 8, 4, S], f16, name=f"h2q{q}")
                    for q in range(2)
                ]

                def emit_h1(i):
                    """xi DMA + h1 matmuls + inline Square/stat-sums."""
                    xi = p1x.tile([P, 8, S], f16, tag="xi", name=f"xi{i}")
                    nc.sync.dma_start(
                        out=xi,
                        in_=xT[:, i * S:(i + 1) * S].rearrange(
                            "(k p) t -> p k t", p=P
                        ),
                    )
                    a0 = p1a.tile([P, 8, S], f16, tag="a0", name=f"a0_{i}")
                    sps0 = psS.tile([P, S], f32, tag="sps0", bufs=1,
                                    name=f"sps0_{i}")
                    sps1 = psS.tile([P, S], f32, tag="sps1", bufs=1,
                                    name=f"sps1_{i}")
                    for mh in range(2):
                        wb = [p1w.tile([P, 512], f16, tag="w",
                                       name=f"w1b{i}_{mh}_{k}")
                              for k in range(8)]
                        for k in range(8):
                            nc.sync.dma_start(
                                out=wb[k],
                                in_=W1s[i, k * P:(k + 1) * P,
                                        mh * 512:(mh + 1) * 512],
                            )
                        for mm in range(4):
                            m = mh * 4 + mm
                            ps = psA.tile([P, S], f32, tag="mm",
                                          name=f"ps1_{i}_{m}")
                            for k in range(8):
                                nc.tensor.matmul(
                                    ps, wb[k][:, mm * P:(mm + 1) * P],
                                    xi[:, k, :],
                                    start=(k == 0), stop=(k == 7),
                                )
                            nc.scalar.activation(
                                out=a0[:, m, :], in_=ps, func=AF.Identity,
                                bias=bcols[:, 0, i * 8 + m: i * 8 + m + 1],
                            )
                            sq = p1a.tile([P, S], f16, tag="sq",
                                          name=f"sq{i}_{m}")
                            nc.scalar.activation(
                                out=sq, in_=a0[:, m, :], func=AF.Square,
                            )
                            nc.tensor.matmul(
                                sps0, onesp, a0[:, m, :],
                                start=(m == 0), stop=(m == 7),
                                skip_group_check=True,
                            )
                            nc.tensor.matmul(
                                sps1, onesp, sq,
                                start=(m == 0), stop=(m == 7),
                                skip_group_check=True,
                            )
                    mrB = p1r.tile([P, 2, S], f32, tag="mrB",
                                   name=f"mrB{i}")
                    nc.scalar.activation(
                        out=mrB[:, 0, :], in_=sps0,
                        func=AF.Identity, scale=1.0 / H,
                    )
                    nc.scalar.activation(
                        out=mrB[:, 1, :], in_=sps1,
                        func=AF.Identity, scale=1.0 / H,
                    )
                    scr = p1r.tile([P, S], f32, tag="scr", name=f"scr{i}")
                    nc.vector.tensor_mul(scr, mrB[:, 0, :], mrB[:, 0, :])
                    nc.vector.tensor_sub(scr, mrB[:, 1, :], scr)
                    # rstd = 1/sqrt(|var| + eps); var >= 0 so same as
                    # rsqrt, and this func shares its act table with
                    # identity/square/relu (no ACT_TABLE_LOAD swaps)
                    nc.scalar.activation(out=mrB[:, 1, :], in_=scr,
                                         func=AF.Abs_reciprocal_sqrt,
                                         bias=eps_sb)
                    return a0, mrB

                def emit_rest(i, a0, mrB):
                    """LN apply + h2 for item i."""
                    a1 = p1a.tile([P, 8, S], f16, tag="a1", name=f"a1_{i}")
                    for m in range(8):
                        nc.vector.tensor_sub(
                            a1[:, m, :], a0[:, m, :], mrB[:, 0, :]
                        )
                        nc.vector.tensor_mul(
                            a1[:, m, :], a1[:, m, :], mrB[:, 1, :]
                        )
                        nc.vector.tensor_scalar(
                            out=a1[:, m, :], in0=a1[:, m, :],
                            scalar1=bcols[:, 1, i * 8 + m: i * 8 + m + 1],
                            scalar2=bcols[:, 2, i * 8 + m: i * 8 + m + 1],
                            op0=ALU.mult, op1=ALU.add,
                        )
                        nc.scalar.activation(
                            out=a1[:, m, :], in_=a1[:, m, :], func=AF.Relu,
                        )

                    q, iq = i // 4, i % 4
                    for mh in range(2):
                        wb = [p1w.tile([P, 512], f16, tag="w",
                                       name=f"w2b{i}_{mh}_{k}")
                              for k in range(8)]
                        for k in range(8):
                            nc.sync.dma_start(
                                out=wb[k],
                                in_=W2s[i, k * P:(k + 1) * P,
                                        mh * 512:(mh + 1) * 512],
                            )
                        for mm in range(4):
                            m = mh * 4 + mm
                            ps = psA.tile([P, S], f32, tag="mm",
                                          name=f"ps2_{i}_{m}")
                            for k in range(8):
                                nc.tensor.matmul(
                                    ps, wb[k][:, mm * P:(mm + 1) * P],
                                    a1[:, k, :],
                                    start=(k == 0), stop=(k == 7),
                                )
                            nc.scalar.activation(
                                out=h2q[q][:, m, iq, :], in_=ps,
                                func=AF.Identity,
                                bias=bcols[:, 3, i * 8 + m: i * 8 + m + 1],
                            )

                def emit_z(q):
                    for c in range(16):
                        for th in range(2):  # 512-token halves of quad
                            zp = psZ.tile([P, 512], f32, tag="zp",
                                          name=f"zp{q}_{c}_{th}")
                            rhs = h2q[q].rearrange("p m i t -> p m (i t)")
                            for k in range(8):
                                nc.tensor.matmul(
                                    zp, wih_sb[:, k, c, :],
                                    rhs[:, k, th * 512:(th + 1) * 512],
                                    start=(k == 0), stop=False,
                                )
                            tsl = slice(q * 1024 + th * 512,
                                        q * 1024 + (th + 1) * 512)
                            nc.tensor.matmul(
                                zp, biasS_sb[:, c * P:(c + 1) * P],
                                mrow_sb[:, tsl],
                                start=False, stop=True,
                            )
                            nc.scalar.activation(
                                out=zT[:, c, tsl], in_=zp,
                                func=AF.Identity,
                            )

                # software pipeline: h1 of item i+1 is emitted before the
                # LN/h2 of item i so the PE FIFO never drains during the
                # LN latency chain
                if 1 in phases:
                    pending = emit_h1(0)
                    for i in range(BC):
                        nxt = emit_h1(i + 1) if i + 1 < BC else None
                        if i == 1:
                            nc.sync.dma_start(
                                out=wih_sb,
                                in_=WihS.rearrange("k c p u -> p k c u"),
                            )
                        emit_rest(i, *pending)
                        pending = nxt
                        if i % 4 == 3:
                            emit_z(i // 4)

            # ================= PHASE 2 =================
            with (
                tc.tile_pool(name="p2whh", bufs=1) as p2whh,
                tc.tile_pool(name="p2s", bufs=1) as p2s,
                tc.tile_pool(name="p2t", bufs=2) as p2t,
                tc.tile_pool(name="psG", bufs=2, space="PSUM") as psG,
            ):
                whh_sb = p2whh.tile([P, 4, 16, P], f16)
                nc.sync.dma_start(
                    out=whh_sb, in_=WhhS.rearrange("k c p u -> p k c u")
                )
                c_st = p2s.tile([P, 4, BC], f32)
                nc.vector.memset(c_st.rearrange("p a b -> p (a b)"), 0.0)
                # zT tokens are item-major (i*S + t); view for per-step
                # strided reads of all 8 items at time s
                zT4 = zT.rearrange("p c (i t) -> p c i t", i=BC)

                for s in range(nsteps if 2 in phases else 0):
                    # one PSUM tile per gate -> independent dep tracking,
                    # so each gate's activation fires right after its own
                    # accumulation group stops (not after the whole burst)
                    gps = [
                        psG.tile([P, 4, BC], f32, tag=f"g{g}",
                                 name=f"gp{g}_{s}")
                        for g in range(4)
                    ]
                    bsl = slice(s * BC, (s + 1) * BC)
                    psl = slice((s - 1) * BC, s * BC)
                    # gate order: 0 g, 1 f, 2 i, 3 o (chunks 4g..4g+3)
                    for g in range(4):
                        csl = slice(4 * g, 4 * g + 4)
                        nc.tensor.matmul(
                            gps[g].rearrange("p c b -> p (c b)"),
                            i128_sb,
                            zT4[:, csl, :, s].rearrange("p c i -> p (c i)"),
                            start=True, stop=(s == 0),
                        )
                        if s > 0:
                            for cc in range(4):
                                c = 4 * g + cc
                                for k in range(4):
                                    nc.tensor.matmul(
                                        gps[g][:, cc, :], whh_sb[:, k, c, :],
                                        ysT[:, k, psl],
                                        start=False,
                                        stop=(cc == 3 and k == 3),
                                        skip_group_check=True,
                                    )
                    sig = p2t.tile([P, 16, BC], f32, tag="sig")
                    nc.scalar.activation(
                        out=sig[:, 0:4, :].rearrange("p c b -> p (c b)"),
                        in_=gps[0].rearrange("p c b -> p (c b)"),
                        func=AF.Tanh,
                    )
                    nc.scalar.activation(
                        out=sig[:, 4:8, :].rearrange("p c b -> p (c b)"),
                        in_=gps[1].rearrange("p c b -> p (c b)"),
                        func=AF.Sigmoid,
                    )
                    nc.scalar.activation(
                        out=sig[:, 8:12, :].rearrange("p c b -> p (c b)"),
                        in_=gps[2].rearrange("p c b -> p (c b)"),
                        func=AF.Sigmoid,
                    )
                    nc.scalar.activation(
                        out=sig[:, 12:16, :].rearrange("p c b -> p (c b)"),
                        in_=gps[3].rearrange("p c b -> p (c b)"),
                        func=AF.Sigmoid,
                    )
                    t1 = p2t.tile([P, 4, BC], f32, tag="t1")
                    nc.vector.tensor_mul(t1, sig[:, 4:8, :], c_st)
                    t2 = p2t.tile([P, 4, BC], f32, tag="t2")
                    nc.vector.tensor_mul(t2, sig[:, 8:12, :], sig[:, 0:4, :])
                    nc.vector.tensor_add(c_st, t1, t2)
                    tc3 = p2t.tile([P, 4, BC], f32, tag="tc")
                    nc.scalar.activation(
                        out=tc3.rearrange("p a b -> p (a b)"),
                        in_=c_st.rearrange("p a b -> p (a b)"),
                        func=AF.Tanh,
                    )
                    nc.vector.tensor_mul(
                        ysT[:, :, bsl], sig[:, 12:16, :], tc3,
                    )

            # ================= PHASE 3 =================
            with (
                tc.tile_pool(name="p3", bufs=4) as p3,
                tc.tile_pool(name="p3w", bufs=1) as p3w,
                tc.tile_pool(name="psP", bufs=4, space="PSUM") as psP,
            ):
                wp_sb = p3w.tile([P, 4, E], f16)
                nc.sync.dma_start(out=wp_sb, in_=WpT)
                for mt in range(TOK // P if 3 in phases else 0):
                    pp = psP.tile([P, E], f32, tag="pp")
                    for k in range(4):
                        nc.tensor.matmul(
                            pp, ysT[:, k, mt * P:(mt + 1) * P],
                            wp_sb[:, k, :],
                            start=(k == 0), stop=(k == 3),
                        )
                    ot = p3.tile([P, E], f32, tag="ot")
                    nc.scalar.activation(out=ot, in_=pp, func=AF.Identity)
                    nc.sync.dma_start(
                        out=partial[mt * P:(mt + 1) * P, :], in_=ot
                    )

    nc.finalize()
    return nc


def _prep_core_inputs(core, perm, seq, am, li, W1, b1, ln_g, ln_b, W2, b2,
                      Wih, Whh, bvec, Wp):
    q = core % 4
    bwd = core >= 4
    items = perm[q * BC:(q + 1) * BC]
    cperm = _chunk_perm()

    x = seq[items]                          # [8, S, H]
    mm = am[items].astype(np.float32)       # [8, S]
    if bwd:
        x = x[:, ::-1, :]
        mm = mm[:, ::-1]
    xT = np.ascontiguousarray(
        x.transpose(2, 0, 1).reshape(H, TOK), dtype=np.float16
    )
    langs = li[items]
    W1s = np.ascontiguousarray(W1[langs], dtype=np.float16)
    W2s = np.ascontiguousarray(W2[langs], dtype=np.float16)

    def cols(v):                            # [L,1024] -> [128, item*8+m]
        vv = v[langs]
        return vv.reshape(BC, 8, P).transpose(2, 0, 1).reshape(P, BC * 8)

    bcols = np.ascontiguousarray(
        np.stack([cols(b1), cols(ln_g), cols(ln_b), cols(b2)], axis=0),
        dtype=np.float32,
    )

    # Wih stationary tiles [k, c, feat128, unit128]: lhsT = Wih.T chunk
    WihP = Wih[cperm, :]                    # [G, H] permuted gate rows
    WihS = np.ascontiguousarray(
        WihP.reshape(16, P, 8, P).transpose(2, 0, 3, 1), dtype=np.float16
    )  # [k, c, feat, unit]
    biasS = np.empty((2, G), dtype=np.float16)
    biasS[0] = bvec[cperm]
    biasS[1] = NEG
    mrow = np.empty((2, TOK), dtype=np.float16)
    mrow[0] = 1.0
    mrow[1] = (mm - 1.0).reshape(TOK)

    WhhP = Whh[cperm, :]                    # [G, HL]
    WhhS = np.ascontiguousarray(
        WhhP.reshape(16, P, 4, P).transpose(2, 0, 3, 1), dtype=np.float16
    )  # [k, c, feat, unit]

    d0 = HL if bwd else 0
    WpT = np.ascontiguousarray(
        Wp[:, d0:d0 + HL].T.reshape(4, P, E).transpose(1, 0, 2),
        dtype=np.float16,
    )  # [p, k, e]

    return {
        "xT": xT, "W1s": W1s, "W2s": W2s, "bcols": bcols,
        "WihS": WihS, "biasS": biasS, "mrow": mrow, "WhhS": WhhS,
        "WpT": WpT, "I128": np.eye(P, dtype=np.float16),
        "OnesP": np.ones((P, P), dtype=np.float16),
    }


def kernel(sequence_output, attention_mask, language_ids, W1, b1, ln_g, ln_b,
           W2, b2, Wih_f, Whh_f, b_f, Wih_b, Whh_b, b_b, Wp, bp):
    from concourse.bass_utils import run_bass_kernel_spmd

    seq = np.asarray(sequence_output, dtype=np.float32)
    am = np.asarray(attention_mask)
    li = np.asarray(language_ids).astype(np.int64)

    key = "nc2"
    if key not in _CACHE:
        _CACHE[key] = _build_nc()
    nc = _CACHE[key]

    perm = np.argsort(li, kind="stable")
    in_maps = []
    for core in range(NCORES):
        bwd = core >= 4
        in_maps.append(
            _prep_core_inputs(
                core, perm, seq, am, li,
                np.asarray(W1, np.float32), np.asarray(b1, np.float32),
                np.asarray(ln_g, np.float32), np.asarray(ln_b, np.float32),
                np.asarray(W2, np.float32), np.asarray(b2, np.float32),
                np.asarray(Wih_b if bwd else Wih_f, np.float32),
                np.asarray(Whh_b if bwd else Whh_f, np.float32),
                np.asarray(b_b if bwd else b_f, np.float32),
                np.asarray(Wp, np.float32),
            )
        )

    trace = bool(os.environ.get("KERNEL_TRACE"))
    res = run_bass_kernel_spmd(
        nc, in_maps, core_ids=list(range(NCORES)), trace=trace
    )
    LAST_RUN["exec_time_ns"] = res.exec_time_ns
    LAST_RUN["profile_json"] = res.profile_json
    # partial rows are ordered (t, b_local): ysT tokens are step-major
    outs = [
        r["partial"].reshape(S, BC, E).transpose(1, 0, 2) for r in res.results
    ]

    out = np.empty((B, S, E), dtype=np.float32)
    bp32 = np.asarray(bp, dtype=np.float32)
    for q in range(4):
        items = perm[q * BC:(q + 1) * BC]
        pf = outs[q]                        # [8, S, E]
        pb = outs[q + 4][:, ::-1, :]        # un-reverse time
        out[items] = pf + pb + bp32
    return out


# revision 4
# speedup vs baseline: 1.1157x; 1.0991x over previous
"""Trainium2 Bass kernel v2 for nn_EntityEncoder (adapters + BiLSTM + proj).

Sharding: 8 cores = 4 batch-quarters x 2 LSTM directions (as v1).

Key changes vs v1:
  - fp16 matmul operands everywhere (1 cyc/col vs fp32r ~3).
  - Phase 2 is weights-stationary: gates land TRANSPOSED [units, batch]
    in PSUM, so elementwise uses all 128 lanes and h' needs no per-step
    PE transpose (its layout is already next step's moving operand).
  - Masking is folded into the gate pre-activations: phase 1 adds
    30*(m-1) to z via a K=2 matmul row, so sigmoid/tanh saturate to
    0/-1 on masked steps. Equivalent to reference retention semantics
    because masks are monotone (fwd: suffix masked; bwd: prefix masked).
  - z kept resident in SBUF as fp16; injected into PSUM via one
    identity matmul per step (no phase-2 DMA at all).

Gate chunk order (unit-chunks of 128 on the partition axis):
  chunks 0-3 = g, 4-7 = i, 8-11 = f, 12-15 = o
so tanh(g) can start earliest and sigma(i,f) = one [128,64]-wide
activation; sigma(o) is last and only feeds the final h-mul.
"""

import os

import numpy as np

B, S, H, HL, E, L = 32, 256, 1024, 512, 256, 5
G = 4 * HL            # 2048 gate width
NCORES = 8
BC = 8                # batch items per core
TOK = BC * S          # tokens per core
EPS = 1e-5
P = 128
NEG = 30.0            # mask kill bias

_CACHE = {}
LAST_RUN = {}

# chunk order on the gate axis: [i, g, f, o] x 4 unit-subchunks
_GATE_OF_CHUNK = [0, 0, 0, 0, 2, 2, 2, 2, 1, 1, 1, 1, 3, 3, 3, 3]


def _chunk_perm():
    """perm[c*128+p] = original gate index for chunk c, unit p.
    Torch gate order in weights: i(0) f(1) g(2) o(3)."""
    perm = np.zeros(G, dtype=np.int64)
    for c in range(16):
        gate = _GATE_OF_CHUNK[c]
        sub = [0, 1, 2, 3][c % 4]
        u = np.arange(128) + sub * 128
        perm[c * 128:(c + 1) * 128] = gate * HL + u
    return perm


def _build_nc(nsteps=S, phases=(1, 2, 3)):
    import concourse.tile as tile
    import concourse.mybir as mybir
    from concourse import bacc

    dt = mybir.dt
    f32 = dt.float32
    f16 = dt.float16
    AF = mybir.ActivationFunctionType
    ALU = mybir.AluOpType

    nc = bacc.Bacc(
        "TRN2", target_bir_lowering=False, debug=False, num_devices=NCORES
    )

    # ---------------- I/O ----------------
    xT = nc.dram_tensor("xT", [H, TOK], f16, kind="ExternalInput").ap()
    W1s = nc.dram_tensor("W1s", [BC, H, H], f16, kind="ExternalInput").ap()
    W2s = nc.dram_tensor("W2s", [BC, H, H], f16, kind="ExternalInput").ap()
    # rows 0..3 are b1, ln_g, ln_b, b2; col = item*8 + feat_chunk
    bcols_d = nc.dram_tensor(
        "bcols", [4, P, BC * 8], f32, kind="ExternalInput"
    ).ap()
    # Wih stationary tiles: [k, c, feat128, unit128] (lhsT per tile)
    WihS = nc.dram_tensor("WihS", [8, 16, P, P], f16, kind="ExternalInput").ap()
    # bias+mask: stationary [2, G] rows (b, NEG*ones); moving [2, TOK]
    # rows (ones, m-1)
    biasS = nc.dram_tensor("biasS", [2, G], f16, kind="ExternalInput").ap()
    mrow = nc.dram_tensor("mrow", [2, TOK], f16, kind="ExternalInput").ap()
    WhhS = nc.dram_tensor("WhhS", [4, 16, P, P], f16, kind="ExternalInput").ap()
    WpT = nc.dram_tensor("WpT", [P, 4, E], f16, kind="ExternalInput").ap()
    I128 = nc.dram_tensor("I128", [P, P], f16, kind="ExternalInput").ap()
    OnesP = nc.dram_tensor("OnesP", [P, P], f16, kind="ExternalInput").ap()
    partial = nc.dram_tensor(
        "partial", [TOK, E], f32, kind="ExternalOutput"
    ).ap()

    with tile.TileContext(nc) as tc:
        with tc.tile_pool(name="persist", bufs=1) as persist:
            bcols = persist.tile([P, 4, BC * 8], f32)
            nc.sync.dma_start(out=bcols, in_=bcols_d.rearrange("s p c -> p s c"))
            i128_sb = persist.tile([P, P], f16)
            nc.sync.dma_start(out=i128_sb, in_=I128)
            onesp = persist.tile([P, P], f16)
            nc.sync.dma_start(out=onesp, in_=OnesP)
            eps_sb = persist.tile([P, 1], f32)
            nc.vector.memset(eps_sb, EPS)

            # z resident in SBUF: [128, chunk, token] fp16
            zT = persist.tile([P, 16, TOK], f16)
            # lstm hidden history, unit-major: [128, k, token] fp16
            ysT = persist.tile([P, 4, TOK], f16)

            # ================= PHASE 1 =================
            with (
                tc.tile_pool(name="p1wih", bufs=1) as p1wih,
                tc.tile_pool(name="p1w", bufs=8) as p1w,
                tc.tile_pool(name="p1misc", bufs=1) as p1misc,
                tc.tile_pool(name="p1x", bufs=2) as p1x,
                tc.tile_pool(name="p1a", bufs=2) as p1a,
                tc.tile_pool(name="p1h2", bufs=1) as p1h2,
                tc.tile_pool(name="p1r", bufs=2) as p1r,
                tc.tile_pool(name="psA", bufs=3, space="PSUM") as psA,
                tc.tile_pool(name="psS", bufs=2, space="PSUM") as psS,
                tc.tile_pool(name="psZ", bufs=2, space="PSUM") as psZ,
            ):
                # Wih stationary tiles in SBUF: [128, k, c, 128]
                # (DMA emitted later, at i==1, so it doesn't block the
                # first items' xi/wb loads in the DMA queues)
                wih_sb = p1wih.tile([P, 8, 16, P], f16)

                mrow_sb = p1misc.tile([2, TOK], f16)
                nc.sync.dma_start(out=mrow_sb, in_=mrow)
                biasS_sb = p1misc.tile([2, G], f16)
                nc.sync.dma_start(out=biasS_sb, in_=biasS)
                # h2 quad buffers: [128, featchunk, item-in-quad, S]
                h2q = [
                    p1h2.tile([P, 8, 4, S], f16, name=f"h2q{q}")
                    for q in range(2)
                ]

                def emit_h1(i):
                    """xi DMA + h1 matmuls + inline Square/stat-sums."""
                    xi = p1x.tile([P, 8, S], f16, tag="xi", name=f"xi{i}")
                    nc.sync.dma_start(
                        out=xi,
                        in_=xT[:, i * S:(i + 1) * S].rearrange(
                            "(k p) t -> p k t", p=P
                        ),
                    )
                    a0 = p1a.tile([P, 8, S], f16, tag="a0", name=f"a0_{i}")
                    sps0 = psS.tile([P, S], f32, tag="sps0", bufs=1,
                                    name=f"sps0_{i}")
                    sps1 = psS.tile([P, S], f32, tag="sps1", bufs=1,
                                    name=f"sps1_{i}")
                    for mh in range(2):
                        wb = [p1w.tile([P, 512], f16, tag="w",
                                       name=f"w1b{i}_{mh}_{k}")
                              for k in range(8)]
                        for k in range(8):
                            nc.sync.dma_start(
                                out=wb[k],
                                in_=W1s[i, k * P:(k + 1) * P,
                                        mh * 512:(mh + 1) * 512],
                            )
                        for mm in range(4):
                            m = mh * 4 + mm
                            ps = psA.tile([P, S], f32, tag="mm",
                                          name=f"ps1_{i}_{m}")
                            for k in range(8):
                                nc.tensor.matmul(
                                    ps, wb[k][:, mm * P:(mm + 1) * P],
                                    xi[:, k, :],
                                    start=(k == 0), stop=(k == 7),
                                )
                            nc.scalar.activation(
                                out=a0[:, m, :], in_=ps, func=AF.Identity,
                                bias=bcols[:, 0, i * 8 + m: i * 8 + m + 1],
                            )
                            sq = p1a.tile([P, S], f16, tag="sq",
                                          name=f"sq{i}_{m}")
                            nc.scalar.activation(
                                out=sq, in_=a0[:, m, :], func=AF.Square,
                            )
                            nc.tensor.matmul(
                                sps0, onesp, a0[:, m, :],
                                start=(m == 0), stop=(m == 7),
                                skip_group_check=True,
                            )
                            nc.tensor.matmul(
                                sps1, onesp, sq,
                                start=(m == 0), stop=(m == 7),
                                skip_group_check=True,
                            )
                    mrB = p1r.tile([P, 2, S], f32, tag="mrB",
                                   name=f"mrB{i}")
                    nc.scalar.activation(
                        out=mrB[:, 0, :], in_=sps0,
                        func=AF.Identity, scale=1.0 / H,
                    )
                    nc.scalar.activation(
                        out=mrB[:, 1, :], in_=sps1,
                        func=AF.Identity, scale=1.0 / H,
                    )
                    scr = p1r.tile([P, S], f32, tag="scr", name=f"scr{i}")
                    nc.vector.tensor_mul(scr, mrB[:, 0, :], mrB[:, 0, :])
                    nc.vector.tensor_sub(scr, mrB[:, 1, :], scr)
                    # rstd = 1/sqrt(|var| + eps); var >= 0 so same as
                    # rsqrt, and this func shares its act table with
                    # identity/square/relu (no ACT_TABLE_LOAD swaps)
                    nc.scalar.activation(out=mrB[:, 1, :], in_=scr,
                                         func=AF.Abs_reciprocal_sqrt,
                                         bias=eps_sb)
                    return a0, mrB

                def emit_rest(i, a0, mrB):
                    """LN apply + h2 for item i."""
                    a1 = p1a.tile([P, 8, S], f16, tag="a1", name=f"a1_{i}")
                    for m in range(8):
                        nc.vector.tensor_sub(
                            a1[:, m, :], a0[:, m, :], mrB[:, 0, :]
                        )
                        nc.vector.tensor_mul(
                            a1[:, m, :], a1[:, m, :], mrB[:, 1, :]
                        )
                        nc.vector.tensor_scalar(
                            out=a1[:, m, :], in0=a1[:, m, :],
                            scalar1=bcols[:, 1, i * 8 + m: i * 8 + m + 1],
                            scalar2=bcols[:, 2, i * 8 + m: i * 8 + m + 1],
                            op0=ALU.mult, op1=ALU.add,
                        )
                        nc.scalar.activation(
                            out=a1[:, m, :], in_=a1[:, m, :], func=AF.Relu,
                        )

                    q, iq = i // 4, i % 4
                    for mh in range(2):
                        wb = [p1w.tile([P, 512], f16, tag="w",
                                       name=f"w2b{i}_{mh}_{k}")
                              for k in range(8)]
                        for k in range(8):
                            nc.sync.dma_start(
                                out=wb[k],
                                in_=W2s[i, k * P:(k + 1) * P,
                                        mh * 512:(mh + 1) * 512],
                            )
                        for mm in range(4):
                            m = mh * 4 + mm
                            ps = psA.tile([P, S], f32, tag="mm",
                                          name=f"ps2_{i}_{m}")
                            for k in range(8):
                                nc.tensor.matmul(
                                    ps, wb[k][:, mm * P:(mm + 1) * P],
                                    a1[:, k, :],
                                    start=(k == 0), stop=(k == 7),
                                )
                            nc.scalar.activation(
                                out=h2q[q][:, m, iq, :], in_=ps,
                                func=AF.Identity,
                                bias=bcols[:, 3, i * 8 + m: i * 8 + m + 1],
                            )

                def emit_z(q):
                    for c in range(16):
                        for th in range(2):  # 512-token halves of quad
                            zp = psZ.tile([P, 512], f32, tag="zp",
                                          name=f"zp{q}_{c}_{th}")
                            rhs = h2q[q].rearrange("p m i t -> p m (i t)")
                            for k in range(8):
                                nc.tensor.matmul(
                                    zp, wih_sb[:, k, c, :],
                                    rhs[:, k, th * 512:(th + 1) * 512],
                                    start=(k == 0), stop=False,
                                )
                            tsl = slice(q * 1024 + th * 512,
                                        q * 1024 + (th + 1) * 512)
                            nc.tensor.matmul(
                                zp, biasS_sb[:, c * P:(c + 1) * P],
                                mrow_sb[:, tsl],
                                start=False, stop=True,
                            )
                            nc.scalar.activation(
                                out=zT[:, c, tsl], in_=zp,
                                func=AF.Identity,
                            )

                # software pipeline: h1 of item i+1 is emitted before the
                # LN/h2 of item i so the PE FIFO never drains during the
                # LN latency chain
                if 1 in phases:
                    pending = emit_h1(0)
                    for i in range(BC):
                        nxt = emit_h1(i + 1) if i + 1 < BC else None
                        if i == 1:
                            nc.sync.dma_start(
                                out=wih_sb,
                                in_=WihS.rearrange("k c p u -> p k c u"),
                            )
                        emit_rest(i, *pending)
                        pending = nxt
                        if i % 4 == 3:
                            emit_z(i // 4)

            # ================= PHASE 2 =================
            with (
                tc.tile_pool(name="p2whh", bufs=1) as p2whh,
                tc.tile_pool(name="p2s", bufs=1) as p2s,
                tc.tile_pool(name="p2t", bufs=2) as p2t,
                tc.tile_pool(name="psG", bufs=2, space="PSUM") as psG,
            ):
                whh_sb = p2whh.tile([P, 4, 16, P], f16)
                nc.sync.dma_start(
                    out=whh_sb, in_=WhhS.rearrange("k c p u -> p k c u")
                )
                c_st = p2s.tile([P, 4, BC], f32)
                nc.vector.memset(c_st.rearrange("p a b -> p (a b)"), 0.0)
                # zT tokens are item-major (i*S + t); view for per-step
                # strided reads of all 8 items at time s
                zT4 = zT.rearrange("p c (i t) -> p c i t", i=BC)

                for s in range(nsteps if 2 in phases else 0):
                    # one PSUM tile per gate -> independent dep tracking,
                    # so each gate's activation fires right after its own
                    # accumulation group stops (not after the whole burst)
                    gps = [
                        psG.tile([P, 4, BC], f32, tag=f"g{g}",
                                 name=f"gp{g}_{s}")
                        for g in range(4)
                    ]
                    bsl = slice(s * BC, (s + 1) * BC)
                    psl = slice((s - 1) * BC, s * BC)
                    # gate order: 0 i, 1 g, 2 f, 3 o (chunks 4g..4g+3)
                    for g in range(4):
                        csl = slice(4 * g, 4 * g + 4)
                        nc.tensor.matmul(
                            gps[g].rearrange("p c b -> p (c b)"),
                            i128_sb,
                            zT4[:, csl, :, s].rearrange("p c i -> p (c i)"),
                            start=True, stop=(s == 0),
                        )
                        if s > 0:
                            for cc in range(4):
                                c = 4 * g + cc
                                for k in range(4):
                                    nc.tensor.matmul(
                                        gps[g][:, cc, :], whh_sb[:, k, c, :],
                                        ysT[:, k, psl],
                                        start=False,
                                        stop=(cc == 3 and k == 3),
                                        skip_group_check=True,
                                    )
                    sig = p2t.tile([P, 16, BC], f32, tag="sig")
                    nc.scalar.activation(
                        out=sig[:, 0:4, :].rearrange("p c b -> p (c b)"),
                        in_=gps[0].rearrange("p c b -> p (c b)"),
                        func=AF.Sigmoid,
                    )
                    nc.scalar.activation(
                        out=sig[:, 4:8, :].rearrange("p c b -> p (c b)"),
                        in_=gps[1].rearrange("p c b -> p (c b)"),
                        func=AF.Tanh,
                    )
                    # t2 = sigma(i) * tanh(g) can run while f/o matmuls go
                    t2 = p2t.tile([P, 4, BC], f32, tag="t2")
                    nc.vector.tensor_mul(t2, sig[:, 0:4, :], sig[:, 4:8, :])
                    nc.scalar.activation(
                        out=sig[:, 8:12, :].rearrange("p c b -> p (c b)"),
                        in_=gps[2].rearrange("p c b -> p (c b)"),
                        func=AF.Sigmoid,
                    )
                    nc.scalar.activation(
                        out=sig[:, 12:16, :].rearrange("p c b -> p (c b)"),
                        in_=gps[3].rearrange("p c b -> p (c b)"),
                        func=AF.Sigmoid,
                    )
                    t1 = p2t.tile([P, 4, BC], f32, tag="t1")
                    nc.vector.tensor_mul(t1, sig[:, 8:12, :], c_st)
                    nc.vector.tensor_add(c_st, t1, t2)
                    tc3 = p2t.tile([P, 4, BC], f32, tag="tc")
                    nc.scalar.activation(
                        out=tc3.rearrange("p a b -> p (a b)"),
                        in_=c_st.rearrange("p a b -> p (a b)"),
                        func=AF.Tanh,
                    )
                    nc.vector.tensor_mul(
                        ysT[:, :, bsl], sig[:, 12:16, :], tc3,
                    )

            # ================= PHASE 3 =================
            with (
                tc.tile_pool(name="p3", bufs=4) as p3,
                tc.tile_pool(name="p3w", bufs=1) as p3w,
                tc.tile_pool(name="psP", bufs=4, space="PSUM") as psP,
            ):
                wp_sb = p3w.tile([P, 4, E], f16)
                nc.sync.dma_start(out=wp_sb, in_=WpT)
                for mt in range(TOK // P if 3 in phases else 0):
                    pp = psP.tile([P, E], f32, tag="pp")
                    for k in range(4):
                        nc.tensor.matmul(
                            pp, ysT[:, k, mt * P:(mt + 1) * P],
                            wp_sb[:, k, :],
                            start=(k == 0), stop=(k == 3),
                        )
                    ot = p3.tile([P, E], f32, tag="ot")
                    nc.scalar.activation(out=ot, in_=pp, func=AF.Identity)
                    nc.sync.dma_start(
                        out=partial[mt * P:(mt + 1) * P, :], in_=ot
                    )

    nc.finalize()
    return nc


def _prep_core_inputs(core, perm, seq, am, li, W1, b1, ln_g, ln_b, W2, b2,
                      Wih, Whh, bvec, Wp):
    q = core % 4
    bwd = core >= 4
    items = perm[q * BC:(q + 1) * BC]
    cperm = _chunk_perm()

    x = seq[items]                          # [8, S, H]
    mm = am[items].astype(np.float32)       # [8, S]
    if bwd:
        x = x[:, ::-1, :]
        mm = mm[:, ::-1]
    xT = np.ascontiguousarray(
        x.transpose(2, 0, 1).reshape(H, TOK), dtype=np.float16
    )
    langs = li[items]
    W1s = np.ascontiguousarray(W1[langs], dtype=np.float16)
    W2s = np.ascontiguousarray(W2[langs], dtype=np.float16)

    def cols(v):                            # [L,1024] -> [128, item*8+m]
        vv = v[langs]
        return vv.reshape(BC, 8, P).transpose(2, 0, 1).reshape(P, BC * 8)

    bcols = np.ascontiguousarray(
        np.stack([cols(b1), cols(ln_g), cols(ln_b), cols(b2)], axis=0),
        dtype=np.float32,
    )

    # Wih stationary tiles [k, c, feat128, unit128]: lhsT = Wih.T chunk
    WihP = Wih[cperm, :]                    # [G, H] permuted gate rows
    WihS = np.ascontiguousarray(
        WihP.reshape(16, P, 8, P).transpose(2, 0, 3, 1), dtype=np.float16
    )  # [k, c, feat, unit]
    biasS = np.empty((2, G), dtype=np.float16)
    biasS[0] = bvec[cperm]
    biasS[1] = NEG
    mrow = np.empty((2, TOK), dtype=np.float16)
    mrow[0] = 1.0
    mrow[1] = (mm - 1.0).reshape(TOK)

    WhhP = Whh[cperm, :]                    # [G, HL]
    WhhS = np.ascontiguousarray(
        WhhP.reshape(16, P, 4, P).transpose(2, 0, 3, 1), dtype=np.float16
    )  # [k, c, feat, unit]

    d0 = HL if bwd else 0
    WpT = np.ascontiguousarray(
        Wp[:, d0:d0 + HL].T.reshape(4, P, E).transpose(1, 0, 2),
        dtype=np.float16,
    )  # [p, k, e]

    return {
        "xT": xT, "W1s": W1s, "W2s": W2s, "bcols": bcols,
        "WihS": WihS, "biasS": biasS, "mrow": mrow, "WhhS": WhhS,
        "WpT": WpT, "I128": np.eye(P, dtype=np.float16),
        "OnesP": np.ones((P, P), dtype=np.float16),
    }


def kernel(sequence_output, attention_mask, language_ids, W1, b1, ln_g, ln_b,
           W2, b2, Wih_f, Whh_f, b_f, Wih_b, Whh_b, b_b, Wp, bp):
    from concourse.bass_utils import run_bass_kernel_spmd

    seq = np.asarray(sequence_output, dtype=np.float32)
    am = np.asarray(attention_mask)
    li = np.asarray(language_ids).astype(np.int64)

    key = "nc2"
    if key not in _CACHE:
        _CACHE[key] = _build_nc()
    nc = _CACHE[key]

    perm = np.argsort(li, kind="stable")
    in_maps = []
    for core in range(NCORES):
        bwd = core >= 4
        in_maps.append(
            _prep_core_inputs(
                core, perm, seq, am, li,
                np.asarray(W1, np.float32), np.asarray(b1, np.float32),
                np.asarray(ln_g, np.float32), np.asarray(ln_b, np.float32),
                np.asarray(W2, np.float32), np.asarray(b2, np.float32),
                np.asarray(Wih_b if bwd else Wih_f, np.float32),
                np.asarray(Whh_b if bwd else Whh_f, np.float32),
                np.asarray(b_b if bwd else b_f, np.float32),
                np.asarray(Wp, np.float32),
            )
        )

    trace = bool(os.environ.get("KERNEL_TRACE"))
    res = run_bass_kernel_spmd(
        nc, in_maps, core_ids=list(range(NCORES)), trace=trace
    )
    LAST_RUN["exec_time_ns"] = res.exec_time_ns
    LAST_RUN["profile_json"] = res.profile_json
    # partial rows are ordered (t, b_local): ysT tokens are step-major
    outs = [
        r["partial"].reshape(S, BC, E).transpose(1, 0, 2) for r in res.results
    ]

    out = np.empty((B, S, E), dtype=np.float32)
    bp32 = np.asarray(bp, dtype=np.float32)
    for q in range(4):
        items = perm[q * BC:(q + 1) * BC]
        pf = outs[q]                        # [8, S, E]
        pb = outs[q + 4][:, ::-1, :]        # un-reverse time
        out[items] = pf + pb + bp32
    return out


# revision 5
# speedup vs baseline: 1.1250x; 1.0083x over previous
"""Trainium2 Bass kernel v2 for nn_EntityEncoder (adapters + BiLSTM + proj).

Sharding: 8 cores = 4 batch-quarters x 2 LSTM directions (as v1).

Key changes vs v1:
  - fp16 matmul operands everywhere (1 cyc/col vs fp32r ~3).
  - Phase 2 is weights-stationary: gates land TRANSPOSED [units, batch]
    in PSUM, so elementwise uses all 128 lanes and h' needs no per-step
    PE transpose (its layout is already next step's moving operand).
  - Masking is folded into the gate pre-activations: phase 1 adds
    30*(m-1) to z via a K=2 matmul row, so sigmoid/tanh saturate to
    0/-1 on masked steps. Equivalent to reference retention semantics
    because masks are monotone (fwd: suffix masked; bwd: prefix masked).
  - z kept resident in SBUF as fp16; injected into PSUM via one
    identity matmul per step (no phase-2 DMA at all).

Gate chunk order (unit-chunks of 128 on the partition axis):
  chunks 0-3 = g, 4-7 = i, 8-11 = f, 12-15 = o
so tanh(g) can start earliest and sigma(i,f) = one [128,64]-wide
activation; sigma(o) is last and only feeds the final h-mul.
"""

import os

import numpy as np

B, S, H, HL, E, L = 32, 256, 1024, 512, 256, 5
G = 4 * HL            # 2048 gate width
NCORES = 8
BC = 8                # batch items per core
TOK = BC * S          # tokens per core
EPS = 1e-5
P = 128
NEG = 30.0            # mask kill bias

_CACHE = {}
LAST_RUN = {}

# chunk order on the gate axis: [i, g, f, o] x 4 unit-subchunks
_GATE_OF_CHUNK = [0, 0, 0, 0, 2, 2, 2, 2, 1, 1, 1, 1, 3, 3, 3, 3]


def _chunk_perm():
    """perm[c*128+p] = original gate index for chunk c, unit p.
    Torch gate order in weights: i(0) f(1) g(2) o(3)."""
    perm = np.zeros(G, dtype=np.int64)
    for c in range(16):
        gate = _GATE_OF_CHUNK[c]
        sub = [0, 1, 2, 3][c % 4]
        u = np.arange(128) + sub * 128
        perm[c * 128:(c + 1) * 128] = gate * HL + u
    return perm


def _build_nc(nsteps=S, phases=(1, 2, 3)):
    import concourse.tile as tile
    import concourse.mybir as mybir
    from concourse import bacc

    dt = mybir.dt
    f32 = dt.float32
    f16 = dt.float16
    AF = mybir.ActivationFunctionType
    ALU = mybir.AluOpType

    nc = bacc.Bacc(
        "TRN2", target_bir_lowering=False, debug=False, num_devices=NCORES
    )

    # ---------------- I/O ----------------
    xT = nc.dram_tensor("xT", [H, TOK], f16, kind="ExternalInput").ap()
    W1s = nc.dram_tensor("W1s", [BC, H, H], f16, kind="ExternalInput").ap()
    W2s = nc.dram_tensor("W2s", [BC, H, H], f16, kind="ExternalInput").ap()
    # rows 0..3 are b1, ln_g, ln_b, b2; col = item*8 + feat_chunk
    bcols_d = nc.dram_tensor(
        "bcols", [4, P, BC * 8], f32, kind="ExternalInput"
    ).ap()
    # Wih stationary tiles: [k, c, feat128, unit128] (lhsT per tile)
    WihS = nc.dram_tensor("WihS", [8, 16, P, P], f16, kind="ExternalInput").ap()
    # bias+mask: stationary [2, G] rows (b, NEG*ones); moving [2, TOK]
    # rows (ones, m-1)
    biasS = nc.dram_tensor("biasS", [2, G], f16, kind="ExternalInput").ap()
    mrow = nc.dram_tensor("mrow", [2, TOK], f16, kind="ExternalInput").ap()
    WhhS = nc.dram_tensor("WhhS", [4, 16, P, P], f16, kind="ExternalInput").ap()
    WpT = nc.dram_tensor("WpT", [P, 4, E], f16, kind="ExternalInput").ap()
    I128 = nc.dram_tensor("I128", [P, P], f16, kind="ExternalInput").ap()
    OnesP = nc.dram_tensor("OnesP", [P, P], f16, kind="ExternalInput").ap()
    partial = nc.dram_tensor(
        "partial", [TOK, E], f32, kind="ExternalOutput"
    ).ap()

    with tile.TileContext(nc) as tc:
        with tc.tile_pool(name="persist", bufs=1) as persist:
            bcols = persist.tile([P, 4, BC * 8], f32)
            nc.sync.dma_start(out=bcols, in_=bcols_d.rearrange("s p c -> p s c"))
            i128_sb = persist.tile([P, P], f16)
            nc.sync.dma_start(out=i128_sb, in_=I128)
            onesp = persist.tile([P, P], f16)
            nc.sync.dma_start(out=onesp, in_=OnesP)
            eps_sb = persist.tile([P, 1], f32)
            nc.vector.memset(eps_sb, EPS)

            # z resident in SBUF: [128, chunk, token] fp16
            zT = persist.tile([P, 16, TOK], f16)
            # lstm hidden history, unit-major: [128, k, token] fp16
            ysT = persist.tile([P, 4, TOK], f16)

            # ================= PHASE 1 =================
            with (
                tc.tile_pool(name="p1wih", bufs=1) as p1wih,
                tc.tile_pool(name="p1w", bufs=4) as p1w,
                tc.tile_pool(name="p1misc", bufs=1) as p1misc,
                tc.tile_pool(name="p1x", bufs=2) as p1x,
                tc.tile_pool(name="p1a", bufs=2) as p1a,
                tc.tile_pool(name="p1h2", bufs=1) as p1h2,
                tc.tile_pool(name="p1r", bufs=2) as p1r,
                tc.tile_pool(name="psA", bufs=3, space="PSUM") as psA,
                tc.tile_pool(name="psS", bufs=2, space="PSUM") as psS,
                tc.tile_pool(name="psZ", bufs=2, space="PSUM") as psZ,
            ):
                # Wih stationary tiles in SBUF: [128, k, c, 128]
                # (DMA emitted later, at i==1, so it doesn't block the
                # first items' xi/wb loads in the DMA queues)
                wih_sb = p1wih.tile([P, 8, 16, P], f16)

                mrow_sb = p1misc.tile([2, TOK], f16)
                nc.sync.dma_start(out=mrow_sb, in_=mrow)
                biasS_sb = p1misc.tile([2, G], f16)
                nc.sync.dma_start(out=biasS_sb, in_=biasS)
                # h2 quad buffers: [128, featchunk, item-in-quad, S]
                h2q = [
                    p1h2.tile([P, 8, 4, S], f16, name=f"h2q{q}")
                    for q in range(2)
                ]

                def emit_h1(i):
                    """xi DMA + h1 matmuls + inline Square/stat-sums."""
                    xi = p1x.tile([P, 8, S], f16, tag="xi", name=f"xi{i}")
                    nc.sync.dma_start(
                        out=xi,
                        in_=xT[:, i * S:(i + 1) * S].rearrange(
                            "(k p) t -> p k t", p=P
                        ),
                    )
                    a0 = p1a.tile([P, 8, S], f16, tag="a0", name=f"a0_{i}")
                    sps0 = psS.tile([P, S], f32, tag="sps0", bufs=1,
                                    name=f"sps0_{i}")
                    sps1 = psS.tile([P, S], f32, tag="sps1", bufs=1,
                                    name=f"sps1_{i}")
                    for q4 in range(4):
                        wb = p1w.tile([P, 8, 256], f16, tag="w",
                                      name=f"w1b{i}_{q4}")
                        nc.sync.dma_start(
                            out=wb,
                            in_=W1s[i, :, q4 * 256:(q4 + 1) * 256].rearrange(
                                "(k p) m -> p k m", p=P
                            ),
                        )
                        for mm in range(2):
                            m = q4 * 2 + mm
                            ps = psA.tile([P, S], f32, tag="mm",
                                          name=f"ps1_{i}_{m}")
                            for k in range(8):
                                nc.tensor.matmul(
                                    ps, wb[:, k, mm * P:(mm + 1) * P],
                                    xi[:, k, :],
                                    start=(k == 0), stop=(k == 7),
                                )
                            nc.scalar.activation(
                                out=a0[:, m, :], in_=ps, func=AF.Identity,
                                bias=bcols[:, 0, i * 8 + m: i * 8 + m + 1],
                            )
                            sq = p1a.tile([P, S], f16, tag="sq",
                                          name=f"sq{i}_{m}")
                            nc.scalar.activation(
                                out=sq, in_=a0[:, m, :], func=AF.Square,
                            )
                            nc.tensor.matmul(
                                sps0, onesp, a0[:, m, :],
                                start=(m == 0), stop=(m == 7),
                                skip_group_check=True,
                            )
                            nc.tensor.matmul(
                                sps1, onesp, sq,
                                start=(m == 0), stop=(m == 7),
                                skip_group_check=True,
                            )
                    mrB = p1r.tile([P, 2, S], f32, tag="mrB",
                                   name=f"mrB{i}")
                    nc.scalar.activation(
                        out=mrB[:, 0, :], in_=sps0,
                        func=AF.Identity, scale=1.0 / H,
                    )
                    nc.scalar.activation(
                        out=mrB[:, 1, :], in_=sps1,
                        func=AF.Identity, scale=1.0 / H,
                    )
                    scr = p1r.tile([P, S], f32, tag="scr", name=f"scr{i}")
                    nc.vector.tensor_mul(scr, mrB[:, 0, :], mrB[:, 0, :])
                    nc.vector.tensor_sub(scr, mrB[:, 1, :], scr)
                    # rstd = 1/sqrt(|var| + eps); var >= 0 so same as
                    # rsqrt, and this func shares its act table with
                    # identity/square/relu (no ACT_TABLE_LOAD swaps)
                    nc.scalar.activation(out=mrB[:, 1, :], in_=scr,
                                         func=AF.Abs_reciprocal_sqrt,
                                         bias=eps_sb)
                    return a0, mrB

                def emit_rest(i, a0, mrB):
                    """LN apply + h2 for item i."""
                    a1 = p1a.tile([P, 8, S], f16, tag="a1", name=f"a1_{i}")
                    for m in range(8):
                        nc.vector.tensor_sub(
                            a1[:, m, :], a0[:, m, :], mrB[:, 0, :]
                        )
                        nc.vector.tensor_mul(
                            a1[:, m, :], a1[:, m, :], mrB[:, 1, :]
                        )
                        nc.vector.tensor_scalar(
                            out=a1[:, m, :], in0=a1[:, m, :],
                            scalar1=bcols[:, 1, i * 8 + m: i * 8 + m + 1],
                            scalar2=bcols[:, 2, i * 8 + m: i * 8 + m + 1],
                            op0=ALU.mult, op1=ALU.add,
                        )
                        nc.scalar.activation(
                            out=a1[:, m, :], in_=a1[:, m, :], func=AF.Relu,
                        )

                    q, iq = i // 4, i % 4
                    for q4 in range(4):
                        wb = p1w.tile([P, 8, 256], f16, tag="w",
                                      name=f"w2b{i}_{q4}")
                        nc.sync.dma_start(
                            out=wb,
                            in_=W2s[i, :, q4 * 256:(q4 + 1) * 256].rearrange(
                                "(k p) m -> p k m", p=P
                            ),
                        )
                        for mm in range(2):
                            m = q4 * 2 + mm
                            ps = psA.tile([P, S], f32, tag="mm",
                                          name=f"ps2_{i}_{m}")
                            for k in range(8):
                                nc.tensor.matmul(
                                    ps, wb[:, k, mm * P:(mm + 1) * P],
                                    a1[:, k, :],
                                    start=(k == 0), stop=(k == 7),
                                )
                            nc.scalar.activation(
                                out=h2q[q][:, m, iq, :], in_=ps,
                                func=AF.Identity,
                                bias=bcols[:, 3, i * 8 + m: i * 8 + m + 1],
                            )

                def emit_z(q):
                    for c in range(16):
                        for th in range(2):  # 512-token halves of quad
                            zp = psZ.tile([P, 512], f32, tag="zp",
                                          name=f"zp{q}_{c}_{th}")
                            rhs = h2q[q].rearrange("p m i t -> p m (i t)")
                            for k in range(8):
                                nc.tensor.matmul(
                                    zp, wih_sb[:, k, c, :],
                                    rhs[:, k, th * 512:(th + 1) * 512],
                                    start=(k == 0), stop=False,
                                )
                            tsl = slice(q * 1024 + th * 512,
                                        q * 1024 + (th + 1) * 512)
                            nc.tensor.matmul(
                                zp, biasS_sb[:, c * P:(c + 1) * P],
                                mrow_sb[:, tsl],
                                start=False, stop=True,
                            )
                            nc.scalar.activation(
                                out=zT[:, c, tsl], in_=zp,
                                func=AF.Identity,
                            )

                # software pipeline: h1 of item i+1 is emitted before the
                # LN/h2 of item i so the PE FIFO never drains during the
                # LN latency chain
                if 1 in phases:
                    pending = emit_h1(0)
                    for i in range(BC):
                        nxt = emit_h1(i + 1) if i + 1 < BC else None
                        if i == 1:
                            nc.sync.dma_start(
                                out=wih_sb,
                                in_=WihS.rearrange("k c p u -> p k c u"),
                            )
                        emit_rest(i, *pending)
                        pending = nxt
                        if i % 4 == 3:
                            emit_z(i // 4)

            # ================= PHASE 2 =================
            with (
                tc.tile_pool(name="p2whh", bufs=1) as p2whh,
                tc.tile_pool(name="p2s", bufs=1) as p2s,
                tc.tile_pool(name="p2t", bufs=2) as p2t,
                tc.tile_pool(name="psG", bufs=2, space="PSUM") as psG,
            ):
                whh_sb = p2whh.tile([P, 4, 16, P], f16)
                nc.sync.dma_start(
                    out=whh_sb, in_=WhhS.rearrange("k c p u -> p k c u")
                )
                c_st = p2s.tile([P, 4, BC], f32)
                nc.vector.memset(c_st.rearrange("p a b -> p (a b)"), 0.0)
                # zT tokens are item-major (i*S + t); view for per-step
                # strided reads of all 8 items at time s
                zT4 = zT.rearrange("p c (i t) -> p c i t", i=BC)

                for s in range(nsteps if 2 in phases else 0):
                    # one PSUM tile per gate -> independent dep tracking,
                    # so each gate's activation fires right after its own
                    # accumulation group stops (not after the whole burst)
                    gps = [
                        psG.tile([P, 4, BC], f32, tag=f"g{g}",
                                 name=f"gp{g}_{s}")
                        for g in range(4)
                    ]
                    bsl = slice(s * BC, (s + 1) * BC)
                    psl = slice((s - 1) * BC, s * BC)
                    # gate order: 0 i, 1 g, 2 f, 3 o (chunks 4g..4g+3)
                    for g in range(4):
                        csl = slice(4 * g, 4 * g + 4)
                        nc.tensor.matmul(
                            gps[g].rearrange("p c b -> p (c b)"),
                            i128_sb,
                            zT4[:, csl, :, s].rearrange("p c i -> p (c i)"),
                            start=True, stop=(s == 0),
                        )
                        if s > 0:
                            for cc in range(4):
                                c = 4 * g + cc
                                for k in range(4):
                                    nc.tensor.matmul(
                                        gps[g][:, cc, :], whh_sb[:, k, c, :],
                                        ysT[:, k, psl],
                                        start=False,
                                        stop=(cc == 3 and k == 3),
                                        skip_group_check=True,
                                    )
                    sig = p2t.tile([P, 16, BC], f32, tag="sig")
                    nc.scalar.activation(
                        out=sig[:, 0:4, :].rearrange("p c b -> p (c b)"),
                        in_=gps[0].rearrange("p c b -> p (c b)"),
                        func=AF.Sigmoid,
                    )
                    nc.scalar.activation(
                        out=sig[:, 4:8, :].rearrange("p c b -> p (c b)"),
                        in_=gps[1].rearrange("p c b -> p (c b)"),
                        func=AF.Tanh,
                    )
                    # t2 = sigma(i) * tanh(g) can run while f/o matmuls go
                    t2 = p2t.tile([P, 4, BC], f32, tag="t2")
                    nc.vector.tensor_mul(t2, sig[:, 0:4, :], sig[:, 4:8, :])
                    nc.scalar.activation(
                        out=sig[:, 8:12, :].rearrange("p c b -> p (c b)"),
                        in_=gps[2].rearrange("p c b -> p (c b)"),
                        func=AF.Sigmoid,
                    )
                    nc.scalar.activation(
                        out=sig[:, 12:16, :].rearrange("p c b -> p (c b)"),
                        in_=gps[3].rearrange("p c b -> p (c b)"),
                        func=AF.Sigmoid,
                    )
                    t1 = p2t.tile([P, 4, BC], f32, tag="t1")
                    nc.vector.tensor_mul(t1, sig[:, 8:12, :], c_st)
                    nc.vector.tensor_add(c_st, t1, t2)
                    tc3 = p2t.tile([P, 4, BC], f32, tag="tc")
                    nc.scalar.activation(
                        out=tc3.rearrange("p a b -> p (a b)"),
                        in_=c_st.rearrange("p a b -> p (a b)"),
                        func=AF.Tanh,
                    )
                    nc.vector.tensor_mul(
                        ysT[:, :, bsl], sig[:, 12:16, :], tc3,
                    )

            # ================= PHASE 3 =================
            with (
                tc.tile_pool(name="p3", bufs=4) as p3,
                tc.tile_pool(name="p3w", bufs=1) as p3w,
                tc.tile_pool(name="psP", bufs=4, space="PSUM") as psP,
            ):
                wp_sb = p3w.tile([P, 4, E], f16)
                nc.sync.dma_start(out=wp_sb, in_=WpT)
                for mt in range(TOK // P if 3 in phases else 0):
                    pp = psP.tile([P, E], f32, tag="pp")
                    for k in range(4):
                        nc.tensor.matmul(
                            pp, ysT[:, k, mt * P:(mt + 1) * P],
                            wp_sb[:, k, :],
                            start=(k == 0), stop=(k == 3),
                        )
                    ot = p3.tile([P, E], f32, tag="ot")
                    nc.scalar.activation(out=ot, in_=pp, func=AF.Identity)
                    nc.sync.dma_start(
                        out=partial[mt * P:(mt + 1) * P, :], in_=ot
                    )

    nc.finalize()
    return nc


def _prep_core_inputs(core, perm, seq, am, li, W1, b1, ln_g, ln_b, W2, b2,
                      Wih, Whh, bvec, Wp):
    q = core % 4
    bwd = core >= 4
    items = perm[q * BC:(q + 1) * BC]
    cperm = _chunk_perm()

    x = seq[items]                          # [8, S, H]
    mm = am[items].astype(np.float32)       # [8, S]
    if bwd:
        x = x[:, ::-1, :]
        mm = mm[:, ::-1]
    xT = np.ascontiguousarray(
        x.transpose(2, 0, 1).reshape(H, TOK), dtype=np.float16
    )
    langs = li[items]
    W1s = np.ascontiguousarray(W1[langs], dtype=np.float16)
    W2s = np.ascontiguousarray(W2[langs], dtype=np.float16)

    def cols(v):                            # [L,1024] -> [128, item*8+m]
        vv = v[langs]
        return vv.reshape(BC, 8, P).transpose(2, 0, 1).reshape(P, BC * 8)

    bcols = np.ascontiguousarray(
        np.stack([cols(b1), cols(ln_g), cols(ln_b), cols(b2)], axis=0),
        dtype=np.float32,
    )

    # Wih stationary tiles [k, c, feat128, unit128]: lhsT = Wih.T chunk
    WihP = Wih[cperm, :]                    # [G, H] permuted gate rows
    WihS = np.ascontiguousarray(
        WihP.reshape(16, P, 8, P).transpose(2, 0, 3, 1), dtype=np.float16
    )  # [k, c, feat, unit]
    biasS = np.empty((2, G), dtype=np.float16)
    biasS[0] = bvec[cperm]
    biasS[1] = NEG
    mrow = np.empty((2, TOK), dtype=np.float16)
    mrow[0] = 1.0
    mrow[1] = (mm - 1.0).reshape(TOK)

    WhhP = Whh[cperm, :]                    # [G, HL]
    WhhS = np.ascontiguousarray(
        WhhP.reshape(16, P, 4, P).transpose(2, 0, 3, 1), dtype=np.float16
    )  # [k, c, feat, unit]

    d0 = HL if bwd else 0
    WpT = np.ascontiguousarray(
        Wp[:, d0:d0 + HL].T.reshape(4, P, E).transpose(1, 0, 2),
        dtype=np.float16,
    )  # [p, k, e]

    return {
        "xT": xT, "W1s": W1s, "W2s": W2s, "bcols": bcols,
        "WihS": WihS, "biasS": biasS, "mrow": mrow, "WhhS": WhhS,
        "WpT": WpT, "I128": np.eye(P, dtype=np.float16),
        "OnesP": np.ones((P, P), dtype=np.float16),
    }


def kernel(sequence_output, attention_mask, language_ids, W1, b1, ln_g, ln_b,
           W2, b2, Wih_f, Whh_f, b_f, Wih_b, Whh_b, b_b, Wp, bp):
    from concourse.bass_utils import run_bass_kernel_spmd

    seq = np.asarray(sequence_output, dtype=np.float32)
    am = np.asarray(attention_mask)
    li = np.asarray(language_ids).astype(np.int64)

    key = "nc2"
    if key not in _CACHE:
        _CACHE[key] = _build_nc()
    nc = _CACHE[key]

    perm = np.argsort(li, kind="stable")
    in_maps = []
    for core in range(NCORES):
        bwd = core >= 4
        in_maps.append(
            _prep_core_inputs(
                core, perm, seq, am, li,
                np.asarray(W1, np.float32), np.asarray(b1, np.float32),
                np.asarray(ln_g, np.float32), np.asarray(ln_b, np.float32),
                np.asarray(W2, np.float32), np.asarray(b2, np.float32),
                np.asarray(Wih_b if bwd else Wih_f, np.float32),
                np.asarray(Whh_b if bwd else Whh_f, np.float32),
                np.asarray(b_b if bwd else b_f, np.float32),
                np.asarray(Wp, np.float32),
            )
        )

    trace = bool(os.environ.get("KERNEL_TRACE"))
    res = run_bass_kernel_spmd(
        nc, in_maps, core_ids=list(range(NCORES)), trace=trace
    )
    LAST_RUN["exec_time_ns"] = res.exec_time_ns
    LAST_RUN["profile_json"] = res.profile_json
    # partial rows are ordered (t, b_local): ysT tokens are step-major
    outs = [
        r["partial"].reshape(S, BC, E).transpose(1, 0, 2) for r in res.results
    ]

    out = np.empty((B, S, E), dtype=np.float32)
    bp32 = np.asarray(bp, dtype=np.float32)
    for q in range(4):
        items = perm[q * BC:(q + 1) * BC]
        pf = outs[q]                        # [8, S, E]
        pb = outs[q + 4][:, ::-1, :]        # un-reverse time
        out[items] = pf + pb + bp32
    return out


# revision 6
# speedup vs baseline: 1.1267x; 1.0015x over previous
"""Trainium2 Bass kernel v2 for nn_EntityEncoder (adapters + BiLSTM + proj).

Sharding: 8 cores = 4 batch-quarters x 2 LSTM directions (as v1).

Key changes vs v1:
  - fp16 matmul operands everywhere (1 cyc/col vs fp32r ~3).
  - Phase 2 is weights-stationary: gates land TRANSPOSED [units, batch]
    in PSUM, so elementwise uses all 128 lanes and h' needs no per-step
    PE transpose (its layout is already next step's moving operand).
  - Masking is folded into the gate pre-activations: phase 1 adds
    30*(m-1) to z via a K=2 matmul row, so sigmoid/tanh saturate to
    0/-1 on masked steps. Equivalent to reference retention semantics
    because masks are monotone (fwd: suffix masked; bwd: prefix masked).
  - z kept resident in SBUF as fp16; injected into PSUM via one
    identity matmul per step (no phase-2 DMA at all).

Gate chunk order (unit-chunks of 128 on the partition axis):
  chunks 0-3 = g, 4-7 = i, 8-11 = f, 12-15 = o
so tanh(g) can start earliest and sigma(i,f) = one [128,64]-wide
activation; sigma(o) is last and only feeds the final h-mul.
"""

import os

import numpy as np

B, S, H, HL, E, L = 32, 256, 1024, 512, 256, 5
G = 4 * HL            # 2048 gate width
NCORES = 8
BC = 8                # batch items per core
TOK = BC * S          # tokens per core
EPS = 1e-5
P = 128
NEG = 30.0            # mask kill bias

_CACHE = {}
LAST_RUN = {}

# chunk order on the gate axis: [i, g, f, o] x 4 unit-subchunks
_GATE_OF_CHUNK = [0, 0, 0, 0, 2, 2, 2, 2, 1, 1, 1, 1, 3, 3, 3, 3]


def _chunk_perm():
    """perm[c*128+p] = original gate index for chunk c, unit p.
    Torch gate order in weights: i(0) f(1) g(2) o(3)."""
    perm = np.zeros(G, dtype=np.int64)
    for c in range(16):
        gate = _GATE_OF_CHUNK[c]
        sub = [0, 1, 2, 3][c % 4]
        u = np.arange(128) + sub * 128
        perm[c * 128:(c + 1) * 128] = gate * HL + u
    return perm


def _build_nc(nsteps=S, phases=(1, 2, 3)):
    import concourse.tile as tile
    import concourse.mybir as mybir
    from concourse import bacc

    dt = mybir.dt
    f32 = dt.float32
    f16 = dt.float16
    AF = mybir.ActivationFunctionType
    ALU = mybir.AluOpType

    nc = bacc.Bacc(
        "TRN2", target_bir_lowering=False, debug=False, num_devices=NCORES
    )

    # ---------------- I/O ----------------
    xT = nc.dram_tensor("xT", [H, TOK], f16, kind="ExternalInput").ap()
    W1s = nc.dram_tensor("W1s", [BC, H, H], f16, kind="ExternalInput").ap()
    W2s = nc.dram_tensor("W2s", [BC, H, H], f16, kind="ExternalInput").ap()
    # rows 0..3 are b1, ln_g, ln_b, b2; col = item*8 + feat_chunk
    bcols_d = nc.dram_tensor(
        "bcols", [4, P, BC * 8], f32, kind="ExternalInput"
    ).ap()
    # Wih stationary tiles: [k, c, feat128, unit128] (lhsT per tile)
    WihS = nc.dram_tensor("WihS", [8, 16, P, P], f16, kind="ExternalInput").ap()
    # bias+mask: stationary [2, G] rows (b, NEG*ones); moving [2, TOK]
    # rows (ones, m-1)
    biasS = nc.dram_tensor("biasS", [2, G], f16, kind="ExternalInput").ap()
    mrow = nc.dram_tensor("mrow", [2, TOK], f16, kind="ExternalInput").ap()
    WhhS = nc.dram_tensor("WhhS", [4, 16, P, P], f16, kind="ExternalInput").ap()
    WpT = nc.dram_tensor("WpT", [P, 4, E], f16, kind="ExternalInput").ap()
    I128 = nc.dram_tensor("I128", [P, P], f16, kind="ExternalInput").ap()
    OnesP = nc.dram_tensor("OnesP", [P, P], f16, kind="ExternalInput").ap()
    partial = nc.dram_tensor(
        "partial", [TOK, E], f32, kind="ExternalOutput"
    ).ap()

    with tile.TileContext(nc) as tc:
        with tc.tile_pool(name="persist", bufs=1) as persist:
            bcols = persist.tile([P, 4, BC * 8], f32)
            nc.sync.dma_start(out=bcols, in_=bcols_d.rearrange("s p c -> p s c"))
            i128_sb = persist.tile([P, P], f16)
            nc.sync.dma_start(out=i128_sb, in_=I128)
            onesp = persist.tile([P, P], f16)
            nc.sync.dma_start(out=onesp, in_=OnesP)
            eps_sb = persist.tile([P, 1], f32)
            nc.vector.memset(eps_sb, EPS)

            # z resident in SBUF: [128, chunk, token] fp16
            zT = persist.tile([P, 16, TOK], f16)
            # lstm hidden history, unit-major: [128, k, token] fp16
            ysT = persist.tile([P, 4, TOK], f16)

            # ================= PHASE 1 =================
            with (
                tc.tile_pool(name="p1wih", bufs=1) as p1wih,
                tc.tile_pool(name="p1w", bufs=5) as p1w,
                tc.tile_pool(name="p1misc", bufs=1) as p1misc,
                tc.tile_pool(name="p1x", bufs=2) as p1x,
                tc.tile_pool(name="p1a", bufs=2) as p1a,
                tc.tile_pool(name="p1h2", bufs=1) as p1h2,
                tc.tile_pool(name="p1r", bufs=2) as p1r,
                tc.tile_pool(name="psA", bufs=3, space="PSUM") as psA,
                tc.tile_pool(name="psS", bufs=2, space="PSUM") as psS,
                tc.tile_pool(name="psZ", bufs=2, space="PSUM") as psZ,
            ):
                # Wih stationary tiles in SBUF: [128, k, c, 128]
                # (DMA emitted later, at i==1, so it doesn't block the
                # first items' xi/wb loads in the DMA queues)
                wih_sb = p1wih.tile([P, 8, 16, P], f16)

                mrow_sb = p1misc.tile([2, TOK], f16)
                nc.sync.dma_start(out=mrow_sb, in_=mrow)
                biasS_sb = p1misc.tile([2, G], f16)
                nc.sync.dma_start(out=biasS_sb, in_=biasS)
                # h2 quad buffers: [128, featchunk, item-in-quad, S]
                h2q = [
                    p1h2.tile([P, 8, 4, S], f16, name=f"h2q{q}")
                    for q in range(2)
                ]

                def emit_h1(i):
                    """xi DMA + h1 matmuls + inline Square/stat-sums."""
                    xi = p1x.tile([P, 8, S], f16, tag="xi", name=f"xi{i}")
                    nc.sync.dma_start(
                        out=xi,
                        in_=xT[:, i * S:(i + 1) * S].rearrange(
                            "(k p) t -> p k t", p=P
                        ),
                    )
                    a0 = p1a.tile([P, 8, S], f16, tag="a0", name=f"a0_{i}")
                    sps0 = psS.tile([P, S], f32, tag="sps0", bufs=1,
                                    name=f"sps0_{i}")
                    sps1 = psS.tile([P, S], f32, tag="sps1", bufs=1,
                                    name=f"sps1_{i}")
                    for q4 in range(4):
                        wb = p1w.tile([P, 8, 256], f16, tag="w",
                                      name=f"w1b{i}_{q4}")
                        nc.sync.dma_start(
                            out=wb,
                            in_=W1s[i, :, q4 * 256:(q4 + 1) * 256].rearrange(
                                "(k p) m -> p k m", p=P
                            ),
                        )
                        for mm in range(2):
                            m = q4 * 2 + mm
                            ps = psA.tile([P, S], f32, tag="mm",
                                          name=f"ps1_{i}_{m}")
                            for k in range(8):
                                nc.tensor.matmul(
                                    ps, wb[:, k, mm * P:(mm + 1) * P],
                                    xi[:, k, :],
                                    start=(k == 0), stop=(k == 7),
                                )
                            nc.scalar.activation(
                                out=a0[:, m, :], in_=ps, func=AF.Identity,
                                bias=bcols[:, 0, i * 8 + m: i * 8 + m + 1],
                            )
                            sq = p1a.tile([P, S], f16, tag="sq",
                                          name=f"sq{i}_{m}")
                            nc.scalar.activation(
                                out=sq, in_=a0[:, m, :], func=AF.Square,
                            )
                            nc.tensor.matmul(
                                sps0, onesp, a0[:, m, :],
                                start=(m == 0), stop=(m == 7),
                                skip_group_check=True,
                            )
                            nc.tensor.matmul(
                                sps1, onesp, sq,
                                start=(m == 0), stop=(m == 7),
                                skip_group_check=True,
                            )
                    mrB = p1r.tile([P, 2, S], f32, tag="mrB",
                                   name=f"mrB{i}")
                    nc.scalar.activation(
                        out=mrB[:, 0, :], in_=sps0,
                        func=AF.Identity, scale=1.0 / H,
                    )
                    nc.scalar.activation(
                        out=mrB[:, 1, :], in_=sps1,
                        func=AF.Identity, scale=1.0 / H,
                    )
                    scr = p1r.tile([P, S], f32, tag="scr", name=f"scr{i}")
                    nc.vector.tensor_mul(scr, mrB[:, 0, :], mrB[:, 0, :])
                    nc.vector.tensor_sub(scr, mrB[:, 1, :], scr)
                    # rstd = 1/sqrt(|var| + eps); var >= 0 so same as
                    # rsqrt, and this func shares its act table with
                    # identity/square/relu (no ACT_TABLE_LOAD swaps)
                    nc.scalar.activation(out=mrB[:, 1, :], in_=scr,
                                         func=AF.Abs_reciprocal_sqrt,
                                         bias=eps_sb)
                    return a0, mrB

                def emit_rest(i, a0, mrB):
                    """LN apply + h2 for item i."""
                    a1 = p1a.tile([P, 8, S], f16, tag="a1", name=f"a1_{i}")
                    for m in range(8):
                        nc.vector.tensor_sub(
                            a1[:, m, :], a0[:, m, :], mrB[:, 0, :]
                        )
                        nc.vector.tensor_mul(
                            a1[:, m, :], a1[:, m, :], mrB[:, 1, :]
                        )
                        nc.vector.tensor_scalar(
                            out=a1[:, m, :], in0=a1[:, m, :],
                            scalar1=bcols[:, 1, i * 8 + m: i * 8 + m + 1],
                            scalar2=bcols[:, 2, i * 8 + m: i * 8 + m + 1],
                            op0=ALU.mult, op1=ALU.add,
                        )
                        nc.scalar.activation(
                            out=a1[:, m, :], in_=a1[:, m, :], func=AF.Relu,
                        )

                    q, iq = i // 4, i % 4
                    for q4 in range(4):
                        wb = p1w.tile([P, 8, 256], f16, tag="w",
                                      name=f"w2b{i}_{q4}")
                        nc.sync.dma_start(
                            out=wb,
                            in_=W2s[i, :, q4 * 256:(q4 + 1) * 256].rearrange(
                                "(k p) m -> p k m", p=P
                            ),
                        )
                        for mm in range(2):
                            m = q4 * 2 + mm
                            ps = psA.tile([P, S], f32, tag="mm",
                                          name=f"ps2_{i}_{m}")
                            for k in range(8):
                                nc.tensor.matmul(
                                    ps, wb[:, k, mm * P:(mm + 1) * P],
                                    a1[:, k, :],
                                    start=(k == 0), stop=(k == 7),
                                )
                            nc.scalar.activation(
                                out=h2q[q][:, m, iq, :], in_=ps,
                                func=AF.Identity,
                                bias=bcols[:, 3, i * 8 + m: i * 8 + m + 1],
                            )

                def emit_z(q):
                    for c in range(16):
                        for th in range(2):  # 512-token halves of quad
                            zp = psZ.tile([P, 512], f32, tag="zp",
                                          name=f"zp{q}_{c}_{th}")
                            rhs = h2q[q].rearrange("p m i t -> p m (i t)")
                            for k in range(8):
                                nc.tensor.matmul(
                                    zp, wih_sb[:, k, c, :],
                                    rhs[:, k, th * 512:(th + 1) * 512],
                                    start=(k == 0), stop=False,
                                )
                            tsl = slice(q * 1024 + th * 512,
                                        q * 1024 + (th + 1) * 512)
                            nc.tensor.matmul(
                                zp, biasS_sb[:, c * P:(c + 1) * P],
                                mrow_sb[:, tsl],
                                start=False, stop=True,
                            )
                            nc.scalar.activation(
                                out=zT[:, c, tsl], in_=zp,
                                func=AF.Identity,
                            )

                # software pipeline: h1 of item i+1 is emitted before the
                # LN/h2 of item i so the PE FIFO never drains during the
                # LN latency chain
                if 1 in phases:
                    pending = emit_h1(0)
                    for i in range(BC):
                        nxt = emit_h1(i + 1) if i + 1 < BC else None
                        if i == 1:
                            nc.sync.dma_start(
                                out=wih_sb,
                                in_=WihS.rearrange("k c p u -> p k c u"),
                            )
                        emit_rest(i, *pending)
                        pending = nxt
                        if i % 4 == 3:
                            emit_z(i // 4)

            # ================= PHASE 2 =================
            with (
                tc.tile_pool(name="p2whh", bufs=1) as p2whh,
                tc.tile_pool(name="p2s", bufs=1) as p2s,
                tc.tile_pool(name="p2t", bufs=2) as p2t,
                tc.tile_pool(name="psG", bufs=2, space="PSUM") as psG,
            ):
                whh_sb = p2whh.tile([P, 4, 16, P], f16)
                # per-k DMAs: step 1's k=0 matmuls unblock after the
                # first quarter instead of the whole 1MB transfer
                for kq in range(4):
                    nc.sync.dma_start(
                        out=whh_sb[:, kq, :, :],
                        in_=WhhS[kq].rearrange("c p u -> p c u"),
                    )
                c_st = p2s.tile([P, 4, BC], f32)
                nc.vector.memset(c_st.rearrange("p a b -> p (a b)"), 0.0)
                # zT tokens are item-major (i*S + t); view for per-step
                # strided reads of all 8 items at time s
                zT4 = zT.rearrange("p c (i t) -> p c i t", i=BC)

                for s in range(nsteps if 2 in phases else 0):
                    # one PSUM tile per gate -> independent dep tracking,
                    # so each gate's activation fires right after its own
                    # accumulation group stops (not after the whole burst)
                    gps = [
                        psG.tile([P, 4, BC], f32, tag=f"g{g}",
                                 name=f"gp{g}_{s}")
                        for g in range(4)
                    ]
                    bsl = slice(s * BC, (s + 1) * BC)
                    psl = slice((s - 1) * BC, s * BC)
                    # gate order: 0 i, 1 g, 2 f, 3 o (chunks 4g..4g+3)
                    for g in range(4):
                        csl = slice(4 * g, 4 * g + 4)
                        nc.tensor.matmul(
                            gps[g].rearrange("p c b -> p (c b)"),
                            i128_sb,
                            zT4[:, csl, :, s].rearrange("p c i -> p (c i)"),
                            start=True, stop=(s == 0),
                        )
                        if s > 0:
                            for cc in range(4):
                                c = 4 * g + cc
                                for k in range(4):
                                    nc.tensor.matmul(
                                        gps[g][:, cc, :], whh_sb[:, k, c, :],
                                        ysT[:, k, psl],
                                        start=False,
                                        stop=(cc == 3 and k == 3),
                                        skip_group_check=True,
                                    )
                    sig = p2t.tile([P, 16, BC], f32, tag="sig")
                    nc.scalar.activation(
                        out=sig[:, 0:4, :].rearrange("p c b -> p (c b)"),
                        in_=gps[0].rearrange("p c b -> p (c b)"),
                        func=AF.Sigmoid,
                    )
                    nc.scalar.activation(
                        out=sig[:, 4:8, :].rearrange("p c b -> p (c b)"),
                        in_=gps[1].rearrange("p c b -> p (c b)"),
                        func=AF.Tanh,
                    )
                    # t2 = sigma(i) * tanh(g) can run while f/o matmuls go
                    t2 = p2t.tile([P, 4, BC], f32, tag="t2")
                    nc.vector.tensor_mul(t2, sig[:, 0:4, :], sig[:, 4:8, :])
                    nc.scalar.activation(
                        out=sig[:, 8:12, :].rearrange("p c b -> p (c b)"),
                        in_=gps[2].rearrange("p c b -> p (c b)"),
                        func=AF.Sigmoid,
                    )
                    nc.scalar.activation(
                        out=sig[:, 12:16, :].rearrange("p c b -> p (c b)"),
                        in_=gps[3].rearrange("p c b -> p (c b)"),
                        func=AF.Sigmoid,
                    )
                    t1 = p2t.tile([P, 4, BC], f32, tag="t1")
                    nc.vector.tensor_mul(t1, sig[:, 8:12, :], c_st)
                    nc.vector.tensor_add(c_st, t1, t2)
                    tc3 = p2t.tile([P, 4, BC], f32, tag="tc")
                    nc.scalar.activation(
                        out=tc3.rearrange("p a b -> p (a b)"),
                        in_=c_st.rearrange("p a b -> p (a b)"),
                        func=AF.Tanh,
                    )
                    nc.vector.tensor_mul(
                        ysT[:, :, bsl], sig[:, 12:16, :], tc3,
                    )

            # ================= PHASE 3 =================
            with (
                tc.tile_pool(name="p3", bufs=4) as p3,
                tc.tile_pool(name="p3w", bufs=1) as p3w,
                tc.tile_pool(name="psP", bufs=4, space="PSUM") as psP,
            ):
                wp_sb = p3w.tile([P, 4, E], f16)
                nc.sync.dma_start(out=wp_sb, in_=WpT)
                for mt in range(TOK // P if 3 in phases else 0):
                    pp = psP.tile([P, E], f32, tag="pp")
                    for k in range(4):
                        nc.tensor.matmul(
                            pp, ysT[:, k, mt * P:(mt + 1) * P],
                            wp_sb[:, k, :],
                            start=(k == 0), stop=(k == 3),
                        )
                    ot = p3.tile([P, E], f32, tag="ot")
                    nc.scalar.activation(out=ot, in_=pp, func=AF.Identity)
                    nc.sync.dma_start(
                        out=partial[mt * P:(mt + 1) * P, :], in_=ot
                    )

    nc.finalize()
    return nc


def _prep_core_inputs(core, perm, seq, am, li, W1, b1, ln_g, ln_b, W2, b2,
                      Wih, Whh, bvec, Wp):
    q = core % 4
    bwd = core >= 4
    items = perm[q * BC:(q + 1) * BC]
    cperm = _chunk_perm()

    x = seq[items]                          # [8, S, H]
    mm = am[items].astype(np.float32)       # [8, S]
    if bwd:
        x = x[:, ::-1, :]
        mm = mm[:, ::-1]
    xT = np.ascontiguousarray(
        x.transpose(2, 0, 1).reshape(H, TOK), dtype=np.float16
    )
    langs = li[items]
    W1s = np.ascontiguousarray(W1[langs], dtype=np.float16)
    W2s = np.ascontiguousarray(W2[langs], dtype=np.float16)

    def cols(v):                            # [L,1024] -> [128, item*8+m]
        vv = v[langs]
        return vv.reshape(BC, 8, P).transpose(2, 0, 1).reshape(P, BC * 8)

    bcols = np.ascontiguousarray(
        np.stack([cols(b1), cols(ln_g), cols(ln_b), cols(b2)], axis=0),
        dtype=np.float32,
    )

    # Wih stationary tiles [k, c, feat128, unit128]: lhsT = Wih.T chunk
    WihP = Wih[cperm, :]                    # [G, H] permuted gate rows
    WihS = np.ascontiguousarray(
        WihP.reshape(16, P, 8, P).transpose(2, 0, 3, 1), dtype=np.float16
    )  # [k, c, feat, unit]
    biasS = np.empty((2, G), dtype=np.float16)
    biasS[0] = bvec[cperm]
    biasS[1] = NEG
    mrow = np.empty((2, TOK), dtype=np.float16)
    mrow[0] = 1.0
    mrow[1] = (mm - 1.0).reshape(TOK)

    WhhP = Whh[cperm, :]                    # [G, HL]
    WhhS = np.ascontiguousarray(
        WhhP.reshape(16, P, 4, P).transpose(2, 0, 3, 1), dtype=np.float16
    )  # [k, c, feat, unit]

    d0 = HL if bwd else 0
    WpT = np.ascontiguousarray(
        Wp[:, d0:d0 + HL].T.reshape(4, P, E).transpose(1, 0, 2),
        dtype=np.float16,
    )  # [p, k, e]

    return {
        "xT": xT, "W1s": W1s, "W2s": W2s, "bcols": bcols,
        "WihS": WihS, "biasS": biasS, "mrow": mrow, "WhhS": WhhS,
        "WpT": WpT, "I128": np.eye(P, dtype=np.float16),
        "OnesP": np.ones((P, P), dtype=np.float16),
    }


def kernel(sequence_output, attention_mask, language_ids, W1, b1, ln_g, ln_b,
           W2, b2, Wih_f, Whh_f, b_f, Wih_b, Whh_b, b_b, Wp, bp):
    from concourse.bass_utils import run_bass_kernel_spmd

    seq = np.asarray(sequence_output, dtype=np.float32)
    am = np.asarray(attention_mask)
    li = np.asarray(language_ids).astype(np.int64)

    key = "nc2"
    if key not in _CACHE:
        _CACHE[key] = _build_nc()
    nc = _CACHE[key]

    perm = np.argsort(li, kind="stable")
    in_maps = []
    for core in range(NCORES):
        bwd = core >= 4
        in_maps.append(
            _prep_core_inputs(
                core, perm, seq, am, li,
                np.asarray(W1, np.float32), np.asarray(b1, np.float32),
                np.asarray(ln_g, np.float32), np.asarray(ln_b, np.float32),
                np.asarray(W2, np.float32), np.asarray(b2, np.float32),
                np.asarray(Wih_b if bwd else Wih_f, np.float32),
                np.asarray(Whh_b if bwd else Whh_f, np.float32),
                np.asarray(b_b if bwd else b_f, np.float32),
                np.asarray(Wp, np.float32),
            )
        )

    trace = bool(os.environ.get("KERNEL_TRACE"))
    res = run_bass_kernel_spmd(
        nc, in_maps, core_ids=list(range(NCORES)), trace=trace
    )
    LAST_RUN["exec_time_ns"] = res.exec_time_ns
    LAST_RUN["profile_json"] = res.profile_json
    # partial rows are ordered (t, b_local): ysT tokens are step-major
    outs = [
        r["partial"].reshape(S, BC, E).transpose(1, 0, 2) for r in res.results
    ]

    out = np.empty((B, S, E), dtype=np.float32)
    bp32 = np.asarray(bp, dtype=np.float32)
    for q in range(4):
        items = perm[q * BC:(q + 1) * BC]
        pf = outs[q]                        # [8, S, E]
        pb = outs[q + 4][:, ::-1, :]        # un-reverse time
        out[items] = pf + pb + bp32
    return out
